# revision 16
# baseline (speedup 1.0000x reference)
"""Trainium2 Bass kernel for the SocialGAN-style decoder (nn_Decoder).

Sharding: data-parallel over scenes. 32 scenes x 24 peds; 8 cores get 4
scenes (96 peds) each, padded to 128 partition lanes (each scene in a
32-lane block, 24 used). Weights are replicated. No cross-core comms.

Algorithmic notes (validated vs reference in fp32 numpy + CoreSim):
  pre_relu[i,j,:] = emb(pos_j - pos_i) @ W_p1[:E] + h_j @ W_p1[E:] + b
                  = u_j - v_i
  with u = pos @ A + h @ Wh1 + b1, v = pos @ A, A = W_pse @ W_p1[:E],
  b1 = b_pse @ W_p1[:E] + b_p1. The (i,j) broadcast grid is emitted as
  PE matmuls against constant selector matrices J (pick j) and -I
  (pick i), so layer 1 of the pool MLP never touches the vector engine.
  relu/max commute: pool = relu(max_j(l2) + b_p2).

Big matmuls run in float32r (full-rate PE, ~2e-4 relative rounding).
fp32r operands must be produced by compute engines (ACT/DVE round on
write); DMA-loaded weights go through a one-time DVE rounding copy.
"""
import numpy as np

import concourse.bacc as bacc
import concourse.mybir as mybir
import concourse.tile as tile
from concourse.bass_utils import run_bass_kernel_spmd

F32 = mybir.dt.float32
F32R = mybir.dt.float32r

# problem dims (hardcoded per contract)
T = 12
E, H, BNK, MLP = 64, 128, 1024, 1024
S, P = 32, 24
B = S * P
N_CORES = 8
SC = S // N_CORES          # scenes per core = 4
PD = 128                   # padded peds per core (4 x 32-lane blocks)
HP = P * P // 2            # pair-columns per half-scene = 288


def build_program(steps=T):
    nc = bacc.Bacc("TRN2", target_bir_lowering=False, debug=False)

    def din(name, shape):
        return nc.dram_tensor(name, list(shape), F32, kind="ExternalInput").ap()

    def dout(name, shape):
        return nc.dram_tensor(name, list(shape), F32, kind="ExternalOutput").ap()

    # per-core sharded state
    h0T_d = din("h0T", (H, PD))
    c0_d = din("c0p", (PD, H))
    lposT_d = din("lposT", (2, PD))
    dec0T_d = din("dec0T", (E, PD))
    # replicated (pre-combined) weights
    wih_d = din("wih", (E + 1, 4 * H))      # [W_ih; b_ih+b_hh]
    whh_d = din("whh", (H, 4 * H))
    whp_d = din("whp", (H, 2))
    bhp_d = din("bhp", (2, 1))
    wse_d = din("wse", (3, E))              # [W_se; b_se]
    aaug_d = din("aaug", (3, 512))          # [A; b1]
    wh1_d = din("wh1", (H, 512))            # W_p1[E:]
    jimat_d = din("jimat", (PD, 2 * HP))    # [J; 0; -I] per 64-block
    wp2_d = din("wp2", (512, BNK))
    bp2_d = din("bp2", (BNK, 1))
    wm1_d = din("wm1", (H + BNK + 1, MLP))  # [W_m1; b_m1]
    wm2_d = din("wm2", (BNK + 1, H))        # [W_m2; b_m2]
    ident_d = din("ident", (PD, PD))
    predT_d = dout("predT", (T, 2, PD))
    hfinT_d = dout("hfinT", (H, PD))

    AF = mybir.ActivationFunctionType
    OP = mybir.AluOpType

    from contextlib import ExitStack
    with tile.TileContext(nc) as tc, ExitStack() as ctx:
        cpool = ctx.enter_context(tc.tile_pool(name="consts", bufs=1))
        spool = ctx.enter_context(tc.tile_pool(name="state", bufs=1))
        stage = ctx.enter_context(tc.tile_pool(name="stage", bufs=2))
        wpool = ctx.enter_context(tc.tile_pool(name="work", bufs=2))
        ypool = ctx.enter_context(tc.tile_pool(name="ysb", bufs=16))
        psA = ctx.enter_context(tc.tile_pool(name="psA", bufs=2, space="PSUM"))
        psB = ctx.enter_context(tc.tile_pool(name="psB", bufs=4, space="PSUM"))
        psC = ctx.enter_context(tc.tile_pool(name="psC", bufs=1, space="PSUM"))

        dma = nc.sync.dma_start

        # ---- load fp32 constants (direct) ----
        def cload(name, ap, dt=F32):
            t = cpool.tile(list(ap.shape), dt, name=name)
            dma(t[:], ap[:])
            return t

        whp = cload("whp", whp_d)
        bhp = cload("bhp", bhp_d)
        wse = cload("wse", wse_d)
        ident = cload("ident", ident_d)
        bp2 = cpool.tile([128, 8], F32, name="bp2")
        dma(bp2[:], bp2_d.rearrange("(m p) one -> p (m one)", p=128))
        wm2 = cpool.tile([128, 8 * H], F32, name="wm2")
        for k in range(8):
            dma(wm2[:, k * H:(k + 1) * H], wm2_d[k * 128:(k + 1) * 128, :])
        wm2b = cpool.tile([1, H], F32, name="wm2b")
        dma(wm2b[:], wm2_d[BNK:BNK + 1, :])

        # ---- fp32r constants: DMA to staging, DVE rounding-copy in ----
        def rload(name, src_ap, chunks=1):
            """Load a DRAM tensor into an F32R tile via staging copies.
            src_ap is (rows<=128, cols); chunks splits cols for staging."""
            rows, cols = src_ap.shape
            t = cpool.tile([rows, cols], F32R, name=name)
            cc = cols // chunks
            for i in range(chunks):
                st = stage.tile([128, 1024], F32, name="st", tag="st")
                dma(st[:rows, :cc], src_ap[:, i * cc:(i + 1) * cc])
                nc.vector.tensor_copy(t[:, i * cc:(i + 1) * cc], st[:rows, :cc])
            return t

        wih = rload("wih", wih_d)
        whh = rload("whh", whh_d)
        aaug = rload("aaug", aaug_d)
        wh1 = rload("wh1", wh1_d)
        jimat = rload("jimat", jimat_d)
        wp2 = cpool.tile([128, 4 * BNK], F32R, name="wp2")  # [k*1024 + m*128]
        for k in range(4):
            st = stage.tile([128, 1024], F32, name="st", tag="st")
            dma(st[:], wp2_d[k * 128:(k + 1) * 128, :])
            nc.vector.tensor_copy(wp2[:, k * BNK:(k + 1) * BNK], st[:])
        wm1 = cpool.tile([128, 9 * MLP], F32R, name="wm1")
        for k in range(9):
            st = stage.tile([128, 1024], F32, name="st", tag="st")
            dma(st[:], wm1_d[k * 128:(k + 1) * 128, :])
            nc.vector.tensor_copy(wm1[:, k * MLP:(k + 1) * MLP], st[:])
        wm1b = cpool.tile([1, MLP], F32R, name="wm1b")
        st = stage.tile([128, 1024], F32, name="st", tag="st")
        dma(st[0:1, :], wm1_d[H + BNK:H + BNK + 1, :])
        nc.vector.tensor_copy(wm1b[:], st[0:1, :])
        ones = cpool.tile([1, PD], F32R, name="ones")
        st = stage.tile([128, 1024], F32, name="st", tag="st")
        nc.vector.memset(st[0:1, :PD], 1.0)
        nc.vector.tensor_copy(ones[:], st[0:1, :PD])

        # ---- state (fp32r where matmul-consumed) ----
        hT = spool.tile([H, PD], F32R, name="hT")         # carry h (transposed)
        st = stage.tile([128, 1024], F32, name="st", tag="st")
        dma(st[:, :PD], h0T_d[:])
        nc.vector.tensor_copy(hT[:], st[:, :PD])
        cst = spool.tile([PD, H], F32, name="cst")        # carry c
        dma(cst[:], c0_d[:])
        posT = spool.tile([3, PD], F32R, name="posT")     # rows 0:2 pos, row 2 ones
        st = stage.tile([128, 1024], F32, name="st", tag="st")
        nc.vector.memset(st[0:3, :PD], 1.0)
        dma(st[0:2, :PD], lposT_d[:])
        nc.vector.tensor_copy(posT[:], st[0:3, :PD])
        decT = spool.tile([E + 1, PD], F32R, name="decT")  # rows 0:E dec, row E ones
        st = stage.tile([128, 1024], F32, name="st", tag="st")
        dma(st[0:E, :PD], dec0T_d[:])
        nc.vector.memset(st[E:E + 1, :PD], 1.0)
        nc.vector.tensor_copy(decT[:], st[0:E + 1, :PD])
        rposT = spool.tile([3, PD], F32, name="rposT")    # rows 0:2 rel, row 2 ones
        st = stage.tile([128, 1024], F32, name="st", tag="st")
        nc.vector.memset(st[0:3, :PD], 1.0)
        nc.vector.tensor_copy(rposT[:], st[0:3, :PD])  # row 2 stays 1.0 forever
        # combined [u; v] per-scene blocks: tile q holds scenes (2q, 2q+1) at
        # rows {0:24 u, 32:56 v, 64:88 u, 96:120 v}; pad rows zeroed once
        # (their JI selector rows are zero, but they must be finite)
        uv = [spool.tile([PD, 512], F32R, name=f"uv{q}") for q in range(2)]
        st = stage.tile([128, 1024], F32, name="st", tag="st")
        nc.vector.memset(st[:, 0:512], 0.0)
        for q in range(2):
            nc.vector.tensor_copy(uv[q][:], st[:, 0:512])
        # full-width relu'd layer-1 tiles (one per contraction chunk):
        # column c = s*576 + h*288 + i_local*24 + j
        ybig = [spool.tile([128, 4 * 576], F32R, name=f"ybig{k}") for k in range(4)]
        # persistent pool output tiles; pad columns zeroed once
        poolT = [spool.tile([128, PD], F32R, name=f"poolT{m}") for m in range(8)]
        stz = stage.tile([128, 1024], F32, name="stz", tag="st")
        nc.vector.memset(stz[:, :PD], 0.0)
        for m in range(8):
            nc.vector.tensor_copy(poolT[m][:], stz[:, :PD])

        f32 = lambda ap: ap.bitcast(F32)

        for t in range(steps):
            # ================= LSTM =================
            gates = psC.tile([PD, 4 * H], F32, tag="c", name="gates")
            nc.tensor.matmul(gates[:], decT[:], wih[:], start=True, stop=False)
            nc.tensor.matmul(gates[:], hT[:], whh[:], start=False, stop=True)
            i_sig = wpool.tile([PD, H], F32, name="i_sig")
            f_sig = wpool.tile([PD, H], F32, name="f_sig")
            g_tan = wpool.tile([PD, H], F32, name="g_tan")
            o_sig = wpool.tile([PD, H], F32, name="o_sig")
            nc.scalar.activation(i_sig[:], gates[:, 0:H], AF.Sigmoid)
            nc.scalar.activation(f_sig[:], gates[:, H:2 * H], AF.Sigmoid)
            nc.scalar.activation(g_tan[:], gates[:, 2 * H:3 * H], AF.Tanh)
            nc.scalar.activation(o_sig[:], gates[:, 3 * H:4 * H], AF.Sigmoid)
            t1 = wpool.tile([PD, H], F32, name="t1")
            nc.vector.tensor_tensor(t1[:], f_sig[:], cst[:], op=OP.mult)
            t2 = wpool.tile([PD, H], F32, name="t2")
            nc.vector.tensor_tensor(t2[:], i_sig[:], g_tan[:], op=OP.mult)
            nc.vector.tensor_tensor(cst[:], t1[:], t2[:], op=OP.add)
            tanh_c = wpool.tile([PD, H], F32, name="tanh_c")
            nc.scalar.activation(tanh_c[:], cst[:], AF.Tanh)
            h_new = wpool.tile([PD, H], F32, name="h_new")
            nc.vector.tensor_tensor(h_new[:], o_sig[:], tanh_c[:], op=OP.mult)
            # h_newT via PE transpose (fp32), rounded to f32r on the ACT copy
            tr0 = psA.tile([PD, PD], F32, tag="y", name="tr0")
            nc.tensor.transpose(tr0[:], h_new[:], ident[:])
            h_newT = wpool.tile([H, PD], F32R, name="h_newT")
            nc.scalar.copy(h_newT[:], tr0[:, :])

            # ================= positions =================
            rp_ps = psC.tile([2, PD], F32, tag="c", name="rp_ps")
            nc.tensor.matmul(rp_ps[:], whp[:], f32(h_newT[:]), start=True, stop=True)
            nc.scalar.activation(rposT[0:2, :], rp_ps[:], AF.Identity, bias=bhp[:])
            dma(predT_d[t], rposT[0:2, :])
            nc.vector.tensor_tensor(posT[0:2, :], f32(posT[0:2, :]),
                                    rposT[0:2, :], op=OP.add)
            dc_ps = psC.tile([E, PD], F32, tag="c", name="dc_ps")
            nc.tensor.matmul(dc_ps[:], wse[:], rposT[:], start=True, stop=True)
            nc.scalar.copy(decT[0:E, :], dc_ps[:])

            # ================= u, v =================
            u_ps = psC.tile([PD, 512], F32, tag="c", name="u_ps")
            nc.tensor.matmul(u_ps[:], posT[:], aaug[:], start=True, stop=False)
            nc.tensor.matmul(u_ps[:], h_newT[:], wh1[:], start=False, stop=True)
            v_ps = psC.tile([PD, 512], F32, tag="c", name="v_ps")
            nc.tensor.matmul(v_ps[:], posT[0:2, :], aaug[0:2, :],
                             start=True, stop=True)
            # scatter u/v scene blocks into the combined uv tiles
            for s in range(SC):
                q, j = divmod(s, 2)
                nc.scalar.copy(uv[q][64 * j:64 * j + P, :],
                               u_ps[32 * s:32 * s + P, :])
                nc.scalar.copy(uv[q][64 * j + 32:64 * j + 32 + P, :],
                               v_ps[32 * s:32 * s + P, :])

            # ================= social pooling =================
            # layer-1 grid: one K=56 matmul per (k-chunk, scene, half)
            for s in range(SC):
                q, j = divmod(s, 2)
                for hh in range(2):
                    for k in range(4):
                        y_ps = psA.tile([128, HP], F32, tag="y", name="y_ps")
                        nc.tensor.matmul(
                            y_ps[:],
                            uv[q][64 * j:64 * j + 56, k * 128:(k + 1) * 128],
                            jimat[64 * j:64 * j + 56, hh * HP:(hh + 1) * HP],
                            start=True, stop=True, tile_position=(64 * j, 0))
                        nc.scalar.activation(
                            ybig[k][:, s * 576 + hh * HP:s * 576 + (hh + 1) * HP],
                            y_ps[:], AF.Relu)
            # layer-2 + grouped max over j: pair columns in 504-wide windows
            praw = [wpool.tile([128, 96], F32, name=f"praw{m}") for m in range(8)]
            NW = [(0, 504), (504, 504), (1008, 504), (1512, 504), (2016, 288)]
            for m in range(8):
                for w, (c0, cw) in enumerate(NW):
                    l2 = psB.tile([128, 504], F32, tag="l2", name="l2")
                    for k in range(4):
                        nc.tensor.matmul(
                            l2[:, 0:cw],
                            wp2[:, k * BNK + m * 128:k * BNK + (m + 1) * 128],
                            ybig[k][:, c0:c0 + cw],
                            start=(k == 0), stop=(k == 3))
                    nc.vector.tensor_reduce(
                        praw[m][:, c0 // P:(c0 + cw) // P],
                        l2[:, 0:cw].rearrange("p (i j) -> p i j", j=P),
                        axis=mybir.AxisListType.X, op=OP.max)
            for m in range(8):
                nc.vector.tensor_scalar(
                    poolT[m][:].rearrange("p (b c) -> p b c", c=32)[:, :, 0:P],
                    praw[m][:].rearrange("p (b c) -> p b c", c=P),
                    bp2[:, m:m + 1], 0.0, op0=OP.add, op1=OP.max)

            # ================= decoder MLP =================
            d1 = psC.tile([PD, MLP], F32, tag="c", name="d1")
            for n in range(2):
                o = d1[:, n * 512:(n + 1) * 512]
                nc.tensor.matmul(o, h_newT[:],
                                 wm1[:, n * 512:n * 512 + 512],
                                 start=True, stop=False)
                for k in range(8):
                    nc.tensor.matmul(
                        o, poolT[k][:],
                        wm1[:, (k + 1) * MLP + n * 512:(k + 1) * MLP + n * 512 + 512],
                        start=False, stop=False)
                nc.tensor.matmul(o, ones[:], wm1b[:, n * 512:(n + 1) * 512],
                                 start=False, stop=True)
            out1 = wpool.tile([PD, MLP], F32, name="out1")
            nc.scalar.activation(out1[:], d1[:], AF.Relu)
            out1T = []
            for k in range(8):
                trk = psA.tile([PD, PD], F32, tag="y", name="trk")
                nc.tensor.transpose(trk[:], out1[:, k * 128:(k + 1) * 128], ident[:])
                o1t = ypool.tile([128, PD], F32, tag="ysb", name="o1t")
                nc.scalar.copy(o1t[:], trk[:])
                out1T.append(o1t)
            d2 = psC.tile([PD, H], F32, tag="c", name="d2")
            for k in range(8):
                nc.tensor.matmul(d2[:], out1T[k][:], wm2[:, k * H:(k + 1) * H],
                                 start=(k == 0), stop=False)
            nc.tensor.matmul(d2[:], f32(ones[:]), wm2b[:], start=False, stop=True)
            hfin = wpool.tile([PD, H], F32, name="hfin")
            nc.scalar.activation(hfin[:], d2[:], AF.Relu)
            trh = psA.tile([PD, PD], F32, tag="y", name="trh")
            nc.tensor.transpose(trh[:], hfin[:], ident[:])
            nc.scalar.copy(hT[:], trh[:, :])

        dma(hfinT_d[:], f32(hT[:]))

    nc.compile()
    return nc


_PROGRAM_CACHE = {}


def _get_program(steps=T):
    if steps not in _PROGRAM_CACHE:
        _PROGRAM_CACHE[steps] = build_program(steps)
    return _PROGRAM_CACHE[steps]


def _host_prep(inputs):
    """Host-side weight precompute + per-core sharding. Returns in_maps."""
    f = lambda k: np.asarray(inputs[k], np.float32)
    W_se, b_se = f("W_se"), f("b_se")
    W_ih, b_ih, W_hh, b_hh = f("W_ih"), f("b_ih"), f("W_hh"), f("b_hh")
    W_hp, b_hp = f("W_hp"), f("b_hp")
    W_pse, b_pse = f("W_pse"), f("b_pse")
    W_p1, b_p1, W_p2, b_p2 = f("W_p1"), f("b_p1"), f("W_p2"), f("b_p2")
    W_m1, b_m1, W_m2, b_m2 = f("W_m1"), f("b_m1"), f("W_m2"), f("b_m2")
    last_pos, last_pos_rel = f("last_pos"), f("last_pos_rel")
    h0, c0 = f("h0"), f("c0")

    A = W_pse @ W_p1[:E]
    b1 = b_pse @ W_p1[:E] + b_p1
    shared = {
        "wih": np.concatenate([W_ih, (b_ih + b_hh)[None]], 0),
        "whh": W_hh,
        "whp": W_hp,
        "bhp": b_hp[:, None].copy(),
        "wse": np.concatenate([W_se, b_se[None]], 0),
        "aaug": np.concatenate([A, b1[None]], 0),
        "wh1": W_p1[E:].copy(),
        "wp2": W_p2,
        "bp2": b_p2[:, None].copy(),
        "wm1": np.concatenate([W_m1, b_m1[None]], 0),
        "wm2": np.concatenate([W_m2, b_m2[None]], 0),
        "ident": np.eye(PD, dtype=np.float32),
    }
    # combined selector matrix: per 64-row block, rows 0:24 pick u_j, rows
    # 32:56 subtract v_i; column c = h*288 + i_local*24 + j
    JI = np.zeros((64, 2 * HP), np.float32)
    for h in range(2):
        for i_l in range(12):
            for jj in range(P):
                JI[jj, h * HP + i_l * P + jj] = 1.0
                JI[32 + 12 * h + i_l, h * HP + i_l * P + jj] = -1.0
    shared["jimat"] = np.concatenate([JI, JI], 0)

    dec0 = last_pos_rel @ W_se + b_se    # (B, E)

    def pad_pedsT(x):  # (n_peds_core, D) -> (D, PD) scene-blocked
        D = x.shape[1]
        out = np.zeros((D, PD), np.float32)
        for s in range(SC):
            out[:, 32 * s:32 * s + P] = x[s * P:(s + 1) * P].T
        return out

    in_maps = []
    for core in range(N_CORES):
        sl = slice(core * SC * P, (core + 1) * SC * P)
        m = dict(shared)
        m["h0T"] = pad_pedsT(h0[0, sl])
        m["c0p"] = pad_pedsT(c0[0, sl]).T.copy()
        m["lposT"] = pad_pedsT(last_pos[sl])[:2].copy()
        m["dec0T"] = pad_pedsT(dec0[sl])
        in_maps.append(m)
    return in_maps


def _unpack(results):
    pred = np.zeros((T, B, 2), np.float32)
    hfin = np.zeros((1, B, H), np.float32)
    for core in range(N_CORES):
        pT = results[core]["predT"]   # (T, 2, PD)
        hT = results[core]["hfinT"]   # (H, PD)
        for s in range(SC):
            gsl = slice((core * SC + s) * P, (core * SC + s + 1) * P)
            psl = slice(32 * s, 32 * s + P)
            pred[:, gsl, :] = pT[:, :, psl].transpose(0, 2, 1)
            hfin[0, gsl, :] = hT[:, psl].T
    return pred, hfin


def kernel(**inputs):
    nc = _get_program()
    in_maps = _host_prep(inputs)
    res = run_bass_kernel_spmd(nc, in_maps, list(range(N_CORES)))
    return _unpack(res.results)


# revision 18
# speedup vs baseline: 1.0175x; 1.0175x over previous
"""Trainium2 Bass kernel for the SocialGAN-style decoder (nn_Decoder).

Sharding: data-parallel over scenes. 32 scenes x 24 peds; 8 cores get 4
scenes (96 peds) each, padded to 128 partition lanes (each scene in a
32-lane block, 24 used). Weights are replicated. No cross-core comms.

Algorithmic notes (validated vs reference in fp32 numpy + CoreSim):
  pre_relu[i,j,:] = emb(pos_j - pos_i) @ W_p1[:E] + h_j @ W_p1[E:] + b
                  = u_j - v_i
  with u = pos @ A + h @ Wh1 + b1, v = pos @ A, A = W_pse @ W_p1[:E],
  b1 = b_pse @ W_p1[:E] + b_p1. The (i,j) broadcast grid is emitted as
  PE matmuls against constant selector matrices J (pick j) and -I
  (pick i), so layer 1 of the pool MLP never touches the vector engine.
  relu/max commute: pool = relu(max_j(l2) + b_p2).

Big matmuls run in float32r (full-rate PE, ~2e-4 relative rounding).
fp32r operands must be produced by compute engines (ACT/DVE round on
write); DMA-loaded weights go through a one-time DVE rounding copy.
"""
import numpy as np

import concourse.bacc as bacc
import concourse.mybir as mybir
import concourse.tile as tile
from concourse.bass_utils import run_bass_kernel_spmd

F32 = mybir.dt.float32
F32R = mybir.dt.float32r

# problem dims (hardcoded per contract)
T = 12
E, H, BNK, MLP = 64, 128, 1024, 1024
S, P = 32, 24
B = S * P
N_CORES = 8
SC = S // N_CORES          # scenes per core = 4
PD = 128                   # padded peds per core (4 x 32-lane blocks)
HP = P * P // 2            # pair-columns per half-scene = 288


def build_program(steps=T):
    nc = bacc.Bacc("TRN2", target_bir_lowering=False, debug=False)

    def din(name, shape):
        return nc.dram_tensor(name, list(shape), F32, kind="ExternalInput").ap()

    def dout(name, shape):
        return nc.dram_tensor(name, list(shape), F32, kind="ExternalOutput").ap()

    # per-core sharded state
    h0T_d = din("h0T", (H, PD))
    c0_d = din("c0p", (PD, H))
    lposT_d = din("lposT", (2, PD))
    dec0T_d = din("dec0T", (E, PD))
    # replicated (pre-combined) weights
    wih_d = din("wih", (E + 1, 4 * H))      # [W_ih; b_ih+b_hh]
    whh_d = din("whh", (H, 4 * H))
    whp_d = din("whp", (H, 2))
    bhp_d = din("bhp", (2, 1))
    wse_d = din("wse", (3, E))              # [W_se; b_se]
    aaug_d = din("aaug", (3, 512))          # [A; b1]
    wh1_d = din("wh1", (H, 512))            # W_p1[E:]
    jimat_d = din("jimat", (PD, 2 * HP))    # [J; 0; -I] per 64-block
    wp2_d = din("wp2", (512, BNK))
    bp2_d = din("bp2", (BNK, 1))
    wm1_d = din("wm1", (H + BNK + 1, MLP))  # [W_m1; b_m1]
    wm2_d = din("wm2", (BNK + 1, H))        # [W_m2; b_m2]
    ident_d = din("ident", (PD, PD))
    predT_d = dout("predT", (T, 2, PD))
    hfinT_d = dout("hfinT", (H, PD))

    AF = mybir.ActivationFunctionType
    OP = mybir.AluOpType

    from contextlib import ExitStack
    with tile.TileContext(nc) as tc, ExitStack() as ctx:
        cpool = ctx.enter_context(tc.tile_pool(name="consts", bufs=1))
        spool = ctx.enter_context(tc.tile_pool(name="state", bufs=1))
        stage = ctx.enter_context(tc.tile_pool(name="stage", bufs=2))
        wpool = ctx.enter_context(tc.tile_pool(name="work", bufs=2))
        ypool = ctx.enter_context(tc.tile_pool(name="ysb", bufs=16))
        psA = ctx.enter_context(tc.tile_pool(name="psA", bufs=2, space="PSUM"))
        psB = ctx.enter_context(tc.tile_pool(name="psB", bufs=5, space="PSUM"))
        psC = ctx.enter_context(tc.tile_pool(name="psC", bufs=1, space="PSUM"))

        dma = nc.sync.dma_start

        # ---- load fp32 constants (direct) ----
        def cload(name, ap, dt=F32):
            t = cpool.tile(list(ap.shape), dt, name=name)
            dma(t[:], ap[:])
            return t

        whp = cload("whp", whp_d)
        bhp = cload("bhp", bhp_d)
        wse = cload("wse", wse_d)
        ident = cload("ident", ident_d)
        bp2 = cpool.tile([128, 8], F32, name="bp2")
        dma(bp2[:], bp2_d.rearrange("(m p) one -> p (m one)", p=128))
        wm2 = cpool.tile([128, 8 * H], F32, name="wm2")
        for k in range(8):
            dma(wm2[:, k * H:(k + 1) * H], wm2_d[k * 128:(k + 1) * 128, :])
        wm2b = cpool.tile([1, H], F32, name="wm2b")
        dma(wm2b[:], wm2_d[BNK:BNK + 1, :])

        # ---- fp32r constants: DMA to staging, DVE rounding-copy in ----
        def rload(name, src_ap, chunks=1):
            """Load a DRAM tensor into an F32R tile via staging copies.
            src_ap is (rows<=128, cols); chunks splits cols for staging."""
            rows, cols = src_ap.shape
            t = cpool.tile([rows, cols], F32R, name=name)
            cc = cols // chunks
            for i in range(chunks):
                st = stage.tile([128, 1024], F32, name="st", tag="st")
                dma(st[:rows, :cc], src_ap[:, i * cc:(i + 1) * cc])
                nc.vector.tensor_copy(t[:, i * cc:(i + 1) * cc], st[:rows, :cc])
            return t

        wih = rload("wih", wih_d)
        whh = rload("whh", whh_d)
        aaug = rload("aaug", aaug_d)
        wh1 = rload("wh1", wh1_d)
        jimat = rload("jimat", jimat_d)
        wp2 = cpool.tile([128, 4 * BNK], F32R, name="wp2")  # [k*1024 + m*128]
        for k in range(4):
            st = stage.tile([128, 1024], F32, name="st", tag="st")
            dma(st[:], wp2_d[k * 128:(k + 1) * 128, :])
            nc.vector.tensor_copy(wp2[:, k * BNK:(k + 1) * BNK], st[:])
        wm1 = cpool.tile([128, 9 * MLP], F32R, name="wm1")
        for k in range(9):
            st = stage.tile([128, 1024], F32, name="st", tag="st")
            dma(st[:], wm1_d[k * 128:(k + 1) * 128, :])
            nc.vector.tensor_copy(wm1[:, k * MLP:(k + 1) * MLP], st[:])
        wm1b = cpool.tile([1, MLP], F32R, name="wm1b")
        st = stage.tile([128, 1024], F32, name="st", tag="st")
        dma(st[0:1, :], wm1_d[H + BNK:H + BNK + 1, :])
        nc.vector.tensor_copy(wm1b[:], st[0:1, :])
        ones = cpool.tile([1, PD], F32R, name="ones")
        st = stage.tile([128, 1024], F32, name="st", tag="st")
        nc.vector.memset(st[0:1, :PD], 1.0)
        nc.vector.tensor_copy(ones[:], st[0:1, :PD])

        # ---- state (fp32r where matmul-consumed) ----
        hT = spool.tile([H, PD], F32R, name="hT")         # carry h (transposed)
        st = stage.tile([128, 1024], F32, name="st", tag="st")
        dma(st[:, :PD], h0T_d[:])
        nc.vector.tensor_copy(hT[:], st[:, :PD])
        cst = spool.tile([PD, H], F32, name="cst")        # carry c
        dma(cst[:], c0_d[:])
        posT = spool.tile([3, PD], F32R, name="posT")     # rows 0:2 pos, row 2 ones
        st = stage.tile([128, 1024], F32, name="st", tag="st")
        nc.vector.memset(st[0:3, :PD], 1.0)
        dma(st[0:2, :PD], lposT_d[:])
        nc.vector.tensor_copy(posT[:], st[0:3, :PD])
        decT = spool.tile([E + 1, PD], F32R, name="decT")  # rows 0:E dec, row E ones
        st = stage.tile([128, 1024], F32, name="st", tag="st")
        dma(st[0:E, :PD], dec0T_d[:])
        nc.vector.memset(st[E:E + 1, :PD], 1.0)
        nc.vector.tensor_copy(decT[:], st[0:E + 1, :PD])
        rposT = spool.tile([3, PD], F32, name="rposT")    # rows 0:2 rel, row 2 ones
        st = stage.tile([128, 1024], F32, name="st", tag="st")
        nc.vector.memset(st[0:3, :PD], 1.0)
        nc.vector.tensor_copy(rposT[:], st[0:3, :PD])  # row 2 stays 1.0 forever
        # combined [u; v] per-scene blocks: tile q holds scenes (2q, 2q+1) at
        # rows {0:24 u, 32:56 v, 64:88 u, 96:120 v}; pad rows zeroed once
        # (their JI selector rows are zero, but they must be finite)
        uv = [spool.tile([PD, 512], F32R, name=f"uv{q}") for q in range(2)]
        st = stage.tile([128, 1024], F32, name="st", tag="st")
        nc.vector.memset(st[:, 0:512], 0.0)
        for q in range(2):
            nc.vector.tensor_copy(uv[q][:], st[:, 0:512])
        # full-width relu'd layer-1 tiles (one per contraction chunk):
        # column c = s*576 + h*288 + i_local*24 + j
        ybig = [spool.tile([128, 4 * 576], F32R, name=f"ybig{k}") for k in range(4)]
        # persistent pool output tiles; pad columns zeroed once
        poolT = [spool.tile([128, PD], F32R, name=f"poolT{m}") for m in range(8)]
        stz = stage.tile([128, 1024], F32, name="stz", tag="st")
        nc.vector.memset(stz[:, :PD], 0.0)
        for m in range(8):
            nc.vector.tensor_copy(poolT[m][:], stz[:, :PD])

        f32 = lambda ap: ap.bitcast(F32)

        for t in range(steps):
            # ================= LSTM =================
            gates = psC.tile([PD, 4 * H], F32, tag="c", name="gates")
            nc.tensor.matmul(gates[:], decT[:], wih[:], start=True, stop=False)
            nc.tensor.matmul(gates[:], hT[:], whh[:], start=False, stop=True)
            i_sig = wpool.tile([PD, H], F32, name="i_sig")
            f_sig = wpool.tile([PD, H], F32, name="f_sig")
            g_tan = wpool.tile([PD, H], F32, name="g_tan")
            o_sig = wpool.tile([PD, H], F32, name="o_sig")
            nc.scalar.activation(i_sig[:], gates[:, 0:H], AF.Sigmoid)
            nc.scalar.activation(f_sig[:], gates[:, H:2 * H], AF.Sigmoid)
            nc.scalar.activation(g_tan[:], gates[:, 2 * H:3 * H], AF.Tanh)
            nc.scalar.activation(o_sig[:], gates[:, 3 * H:4 * H], AF.Sigmoid)
            t1 = wpool.tile([PD, H], F32, name="t1")
            nc.vector.tensor_tensor(t1[:], f_sig[:], cst[:], op=OP.mult)
            t2 = wpool.tile([PD, H], F32, name="t2")
            nc.vector.tensor_tensor(t2[:], i_sig[:], g_tan[:], op=OP.mult)
            nc.vector.tensor_tensor(cst[:], t1[:], t2[:], op=OP.add)
            tanh_c = wpool.tile([PD, H], F32, name="tanh_c")
            nc.scalar.activation(tanh_c[:], cst[:], AF.Tanh)
            h_new = wpool.tile([PD, H], F32, name="h_new")
            nc.vector.tensor_tensor(h_new[:], o_sig[:], tanh_c[:], op=OP.mult)
            # h_newT via PE transpose (fp32), rounded to f32r on the ACT copy
            tr0 = psA.tile([PD, PD], F32, tag="y", name="tr0")
            nc.tensor.transpose(tr0[:], h_new[:], ident[:])
            h_newT = wpool.tile([H, PD], F32R, name="h_newT")
            nc.scalar.copy(h_newT[:], tr0[:, :])

            # ================= positions =================
            rp_ps = psC.tile([2, PD], F32, tag="c", name="rp_ps")
            nc.tensor.matmul(rp_ps[:], whp[:], f32(h_newT[:]), start=True, stop=True)
            nc.scalar.activation(rposT[0:2, :], rp_ps[:], AF.Identity, bias=bhp[:])
            dma(predT_d[t], rposT[0:2, :])
            nc.vector.tensor_tensor(posT[0:2, :], f32(posT[0:2, :]),
                                    rposT[0:2, :], op=OP.add)
            dc_ps = psC.tile([E, PD], F32, tag="c", name="dc_ps")
            nc.tensor.matmul(dc_ps[:], wse[:], rposT[:], start=True, stop=True)
            nc.scalar.copy(decT[0:E, :], dc_ps[:])

            # ================= u, v =================
            u_ps = psC.tile([PD, 512], F32, tag="c", name="u_ps")
            nc.tensor.matmul(u_ps[:], posT[:], aaug[:], start=True, stop=False)
            nc.tensor.matmul(u_ps[:], h_newT[:], wh1[:], start=False, stop=True)
            v_ps = psC.tile([PD, 512], F32, tag="c", name="v_ps")
            nc.tensor.matmul(v_ps[:], posT[0:2, :], aaug[0:2, :],
                             start=True, stop=True)
            # scatter u/v scene blocks into the combined uv tiles
            for s in range(SC):
                q, j = divmod(s, 2)
                nc.scalar.copy(uv[q][64 * j:64 * j + P, :],
                               u_ps[32 * s:32 * s + P, :])
                nc.scalar.copy(uv[q][64 * j + 32:64 * j + 32 + P, :],
                               v_ps[32 * s:32 * s + P, :])

            # ================= social pooling =================
            # layer-1 grid: one K=56 matmul per (k-chunk, scene, half)
            for s in range(SC):
                q, j = divmod(s, 2)
                for hh in range(2):
                    for k in range(4):
                        y_ps = psA.tile([128, HP], F32, tag="y", name="y_ps")
                        nc.tensor.matmul(
                            y_ps[:],
                            uv[q][64 * j:64 * j + 56, k * 128:(k + 1) * 128],
                            jimat[64 * j:64 * j + 56, hh * HP:(hh + 1) * HP],
                            start=True, stop=True, tile_position=(64 * j, 0))
                        nc.scalar.activation(
                            ybig[k][:, s * 576 + hh * HP:s * 576 + (hh + 1) * HP],
                            y_ps[:], AF.Relu)
            # layer-2 + grouped max over j: pair columns in 504-wide windows
            praw = [wpool.tile([128, 96], F32, name=f"praw{m}") for m in range(8)]
            NW = [(0, 504), (504, 504), (1008, 504), (1512, 504), (2016, 288)]
            for m in range(8):
                for w, (c0, cw) in enumerate(NW):
                    l2 = psB.tile([128, 504], F32, tag="l2", name="l2")
                    for k in range(4):
                        nc.tensor.matmul(
                            l2[:, 0:cw],
                            wp2[:, k * BNK + m * 128:k * BNK + (m + 1) * 128],
                            ybig[k][:, c0:c0 + cw],
                            start=(k == 0), stop=(k == 3))
                    nc.vector.tensor_reduce(
                        praw[m][:, c0 // P:(c0 + cw) // P],
                        l2[:, 0:cw].rearrange("p (i j) -> p i j", j=P),
                        axis=mybir.AxisListType.X, op=OP.max)
            for m in range(8):
                nc.vector.tensor_scalar(
                    poolT[m][:].rearrange("p (b c) -> p b c", c=32)[:, :, 0:P],
                    praw[m][:].rearrange("p (b c) -> p b c", c=P),
                    bp2[:, m:m + 1], 0.0, op0=OP.add, op1=OP.max)

            # ================= decoder MLP =================
            out1 = wpool.tile([PD, MLP], F32, name="out1")
            for n in range(2):
                d1 = psC.tile([PD, 512], F32, tag="c", name="d1")
                nc.tensor.matmul(d1[:], h_newT[:],
                                 wm1[:, n * 512:n * 512 + 512],
                                 start=True, stop=False)
                for k in range(8):
                    nc.tensor.matmul(
                        d1[:], poolT[k][:],
                        wm1[:, (k + 1) * MLP + n * 512:(k + 1) * MLP + n * 512 + 512],
                        start=False, stop=False)
                nc.tensor.matmul(d1[:], ones[:], wm1b[:, n * 512:(n + 1) * 512],
                                 start=False, stop=True)
                nc.scalar.activation(out1[:, n * 512:(n + 1) * 512], d1[:], AF.Relu)
            out1T = []
            for k in range(8):
                trk = psA.tile([PD, PD], F32, tag="y", name="trk")
                nc.tensor.transpose(trk[:], out1[:, k * 128:(k + 1) * 128], ident[:])
                o1t = ypool.tile([128, PD], F32, tag="ysb", name="o1t")
                nc.scalar.copy(o1t[:], trk[:])
                out1T.append(o1t)
            d2 = psC.tile([PD, H], F32, tag="c", name="d2")
            for k in range(8):
                nc.tensor.matmul(d2[:], out1T[k][:], wm2[:, k * H:(k + 1) * H],
                                 start=(k == 0), stop=False)
            nc.tensor.matmul(d2[:], f32(ones[:]), wm2b[:], start=False, stop=True)
            hfin = wpool.tile([PD, H], F32, name="hfin")
            nc.scalar.activation(hfin[:], d2[:], AF.Relu)
            trh = psA.tile([PD, PD], F32, tag="y", name="trh")
            nc.tensor.transpose(trh[:], hfin[:], ident[:])
            nc.scalar.copy(hT[:], trh[:, :])

        dma(hfinT_d[:], f32(hT[:]))

    nc.compile()
    return nc


_PROGRAM_CACHE = {}


def _get_program(steps=T):
    if steps not in _PROGRAM_CACHE:
        _PROGRAM_CACHE[steps] = build_program(steps)
    return _PROGRAM_CACHE[steps]


def _host_prep(inputs):
    """Host-side weight precompute + per-core sharding. Returns in_maps."""
    f = lambda k: np.asarray(inputs[k], np.float32)
    W_se, b_se = f("W_se"), f("b_se")
    W_ih, b_ih, W_hh, b_hh = f("W_ih"), f("b_ih"), f("W_hh"), f("b_hh")
    W_hp, b_hp = f("W_hp"), f("b_hp")
    W_pse, b_pse = f("W_pse"), f("b_pse")
    W_p1, b_p1, W_p2, b_p2 = f("W_p1"), f("b_p1"), f("W_p2"), f("b_p2")
    W_m1, b_m1, W_m2, b_m2 = f("W_m1"), f("b_m1"), f("W_m2"), f("b_m2")
    last_pos, last_pos_rel = f("last_pos"), f("last_pos_rel")
    h0, c0 = f("h0"), f("c0")

    A = W_pse @ W_p1[:E]
    b1 = b_pse @ W_p1[:E] + b_p1
    shared = {
        "wih": np.concatenate([W_ih, (b_ih + b_hh)[None]], 0),
        "whh": W_hh,
        "whp": W_hp,
        "bhp": b_hp[:, None].copy(),
        "wse": np.concatenate([W_se, b_se[None]], 0),
        "aaug": np.concatenate([A, b1[None]], 0),
        "wh1": W_p1[E:].copy(),
        "wp2": W_p2,
        "bp2": b_p2[:, None].copy(),
        "wm1": np.concatenate([W_m1, b_m1[None]], 0),
        "wm2": np.concatenate([W_m2, b_m2[None]], 0),
        "ident": np.eye(PD, dtype=np.float32),
    }
    # combined selector matrix: per 64-row block, rows 0:24 pick u_j, rows
    # 32:56 subtract v_i; column c = h*288 + i_local*24 + j
    JI = np.zeros((64, 2 * HP), np.float32)
    for h in range(2):
        for i_l in range(12):
            for jj in range(P):
                JI[jj, h * HP + i_l * P + jj] = 1.0
                JI[32 + 12 * h + i_l, h * HP + i_l * P + jj] = -1.0
    shared["jimat"] = np.concatenate([JI, JI], 0)

    dec0 = last_pos_rel @ W_se + b_se    # (B, E)

    def pad_pedsT(x):  # (n_peds_core, D) -> (D, PD) scene-blocked
        D = x.shape[1]
        out = np.zeros((D, PD), np.float32)
        for s in range(SC):
            out[:, 32 * s:32 * s + P] = x[s * P:(s + 1) * P].T
        return out

    in_maps = []
    for core in range(N_CORES):
        sl = slice(core * SC * P, (core + 1) * SC * P)
        m = dict(shared)
        m["h0T"] = pad_pedsT(h0[0, sl])
        m["c0p"] = pad_pedsT(c0[0, sl]).T.copy()
        m["lposT"] = pad_pedsT(last_pos[sl])[:2].copy()
        m["dec0T"] = pad_pedsT(dec0[sl])
        in_maps.append(m)
    return in_maps


def _unpack(results):
    pred = np.zeros((T, B, 2), np.float32)
    hfin = np.zeros((1, B, H), np.float32)
    for core in range(N_CORES):
        pT = results[core]["predT"]   # (T, 2, PD)
        hT = results[core]["hfinT"]   # (H, PD)
        for s in range(SC):
            gsl = slice((core * SC + s) * P, (core * SC + s + 1) * P)
            psl = slice(32 * s, 32 * s + P)
            pred[:, gsl, :] = pT[:, :, psl].transpose(0, 2, 1)
            hfin[0, gsl, :] = hT[:, psl].T
    return pred, hfin


def kernel(**inputs):
    nc = _get_program()
    in_maps = _host_prep(inputs)
    res = run_bass_kernel_spmd(nc, in_maps, list(range(N_CORES)))
    return _unpack(res.results)


# revision 19
# speedup vs baseline: 1.1036x; 1.0846x over previous
"""Trainium2 Bass kernel for the SocialGAN-style decoder (nn_Decoder).

Sharding: data-parallel over scenes. 32 scenes x 24 peds; 8 cores get 4
scenes (96 peds) each, padded to 128 partition lanes (each scene in a
32-lane block, 24 used). Weights are replicated. No cross-core comms.

Algorithmic notes (validated vs reference in fp32 numpy + CoreSim):
  pre_relu[i,j,:] = emb(pos_j - pos_i) @ W_p1[:E] + h_j @ W_p1[E:] + b
                  = u_j - v_i
  with u = pos @ A + h @ Wh1 + b1, v = pos @ A, A = W_pse @ W_p1[:E],
  b1 = b_pse @ W_p1[:E] + b_p1. The (i,j) broadcast grid is emitted as
  PE matmuls against constant selector matrices J (pick j) and -I
  (pick i), so layer 1 of the pool MLP never touches the vector engine.
  relu/max commute: pool = relu(max_j(l2) + b_p2).

Big matmuls run in float32r (full-rate PE, ~2e-4 relative rounding).
fp32r operands must be produced by compute engines (ACT/DVE round on
write); DMA-loaded weights go through a one-time DVE rounding copy.
"""
import numpy as np

import concourse.bacc as bacc
import concourse.mybir as mybir
import concourse.tile as tile
from concourse.bass_utils import run_bass_kernel_spmd

F32 = mybir.dt.float32
F32R = mybir.dt.float32r

# problem dims (hardcoded per contract)
T = 12
E, H, BNK, MLP = 64, 128, 1024, 1024
S, P = 32, 24
B = S * P
N_CORES = 8
SC = S // N_CORES          # scenes per core = 4
PD = 128                   # padded peds per core (4 x 32-lane blocks)
HP = P * P // 2            # pair-columns per half-scene = 288


def build_program(steps=T):
    nc = bacc.Bacc("TRN2", target_bir_lowering=False, debug=False)

    def din(name, shape):
        return nc.dram_tensor(name, list(shape), F32, kind="ExternalInput").ap()

    def dout(name, shape):
        return nc.dram_tensor(name, list(shape), F32, kind="ExternalOutput").ap()

    # per-core sharded state
    h0T_d = din("h0T", (H, PD))
    c0_d = din("c0p", (PD, H))
    lposT_d = din("lposT", (2, PD))
    dec0T_d = din("dec0T", (E, PD))
    # replicated (pre-combined) weights
    wih_d = din("wih", (E + 1, 4 * H))      # [W_ih; b_ih+b_hh]
    whh_d = din("whh", (H, 4 * H))
    whp_d = din("whp", (H, 2))
    bhp_d = din("bhp", (2, 1))
    wse_d = din("wse", (3, E))              # [W_se; b_se]
    aaug_d = din("aaug", (3, 512))          # [A; b1]
    wh1_d = din("wh1", (H, 512))            # W_p1[E:]
    jimat_d = din("jimat", (PD, 2 * HP))    # [J; 0; -I] per 64-block
    wp2_d = din("wp2", (512, BNK))
    bp2_d = din("bp2", (BNK, 1))
    wm1_d = din("wm1", (H + BNK + 1, MLP))  # [W_m1; b_m1]
    wm2_d = din("wm2", (BNK + 1, H))        # [W_m2; b_m2]
    ident_d = din("ident", (PD, PD))
    predT_d = dout("predT", (T, 2, PD))
    hfinT_d = dout("hfinT", (H, PD))

    AF = mybir.ActivationFunctionType
    OP = mybir.AluOpType

    from contextlib import ExitStack
    with tile.TileContext(nc) as tc, ExitStack() as ctx:
        cpool = ctx.enter_context(tc.tile_pool(name="consts", bufs=1))
        spool = ctx.enter_context(tc.tile_pool(name="state", bufs=1))
        stage = ctx.enter_context(tc.tile_pool(name="stage", bufs=2))
        wpool = ctx.enter_context(tc.tile_pool(name="work", bufs=2))
        ypool = ctx.enter_context(tc.tile_pool(name="ysb", bufs=16))
        ps1 = ctx.enter_context(tc.tile_pool(name="ps1", bufs=4, space="PSUM"))
        psB = ctx.enter_context(tc.tile_pool(name="psB", bufs=4, space="PSUM"))

        dma = nc.sync.dma_start

        # ---- load fp32 constants (direct) ----
        def cload(name, ap, dt=F32):
            t = cpool.tile(list(ap.shape), dt, name=name)
            dma(t[:], ap[:])
            return t

        whp = cload("whp", whp_d)
        bhp = cload("bhp", bhp_d)
        wse = cload("wse", wse_d)
        ident = cload("ident", ident_d)
        bp2 = cpool.tile([128, 8], F32, name="bp2")
        dma(bp2[:], bp2_d.rearrange("(m p) one -> p (m one)", p=128))
        wm2 = cpool.tile([128, 8 * H], F32, name="wm2")
        for k in range(8):
            dma(wm2[:, k * H:(k + 1) * H], wm2_d[k * 128:(k + 1) * 128, :])
        wm2b = cpool.tile([1, H], F32, name="wm2b")
        dma(wm2b[:], wm2_d[BNK:BNK + 1, :])

        # ---- fp32r constants: DMA to staging, DVE rounding-copy in ----
        def rload(name, src_ap, chunks=1):
            """Load a DRAM tensor into an F32R tile via staging copies.
            src_ap is (rows<=128, cols); chunks splits cols for staging."""
            rows, cols = src_ap.shape
            t = cpool.tile([rows, cols], F32R, name=name)
            cc = cols // chunks
            for i in range(chunks):
                st = stage.tile([128, 1024], F32, name="st", tag="st")
                dma(st[:rows, :cc], src_ap[:, i * cc:(i + 1) * cc])
                nc.vector.tensor_copy(t[:, i * cc:(i + 1) * cc], st[:rows, :cc])
            return t

        wih = rload("wih", wih_d)
        whh = rload("whh", whh_d)
        aaug = rload("aaug", aaug_d)
        wh1 = rload("wh1", wh1_d)
        jimat = rload("jimat", jimat_d)
        wp2 = cpool.tile([128, 4 * BNK], F32R, name="wp2")  # [k*1024 + m*128]
        for k in range(4):
            st = stage.tile([128, 1024], F32, name="st", tag="st")
            dma(st[:], wp2_d[k * 128:(k + 1) * 128, :])
            nc.vector.tensor_copy(wp2[:, k * BNK:(k + 1) * BNK], st[:])
        wm1 = cpool.tile([128, 9 * MLP], F32R, name="wm1")
        for k in range(9):
            st = stage.tile([128, 1024], F32, name="st", tag="st")
            dma(st[:], wm1_d[k * 128:(k + 1) * 128, :])
            nc.vector.tensor_copy(wm1[:, k * MLP:(k + 1) * MLP], st[:])
        wm1b = cpool.tile([1, MLP], F32R, name="wm1b")
        st = stage.tile([128, 1024], F32, name="st", tag="st")
        dma(st[0:1, :], wm1_d[H + BNK:H + BNK + 1, :])
        nc.vector.tensor_copy(wm1b[:], st[0:1, :])
        ones = cpool.tile([1, PD], F32R, name="ones")
        st = stage.tile([128, 1024], F32, name="st", tag="st")
        nc.vector.memset(st[0:1, :PD], 1.0)
        nc.vector.tensor_copy(ones[:], st[0:1, :PD])

        # ---- state (fp32r where matmul-consumed) ----
        hT = spool.tile([H, PD], F32R, name="hT")         # carry h (transposed)
        st = stage.tile([128, 1024], F32, name="st", tag="st")
        dma(st[:, :PD], h0T_d[:])
        nc.vector.tensor_copy(hT[:], st[:, :PD])
        cst = spool.tile([PD, H], F32, name="cst")        # carry c
        dma(cst[:], c0_d[:])
        posT = spool.tile([3, PD], F32R, name="posT")     # rows 0:2 pos, row 2 ones
        st = stage.tile([128, 1024], F32, name="st", tag="st")
        nc.vector.memset(st[0:3, :PD], 1.0)
        dma(st[0:2, :PD], lposT_d[:])
        nc.vector.tensor_copy(posT[:], st[0:3, :PD])
        decT = spool.tile([E + 1, PD], F32R, name="decT")  # rows 0:E dec, row E ones
        st = stage.tile([128, 1024], F32, name="st", tag="st")
        dma(st[0:E, :PD], dec0T_d[:])
        nc.vector.memset(st[E:E + 1, :PD], 1.0)
        nc.vector.tensor_copy(decT[:], st[0:E + 1, :PD])
        rposT = spool.tile([3, PD], F32, name="rposT")    # rows 0:2 rel, row 2 ones
        st = stage.tile([128, 1024], F32, name="st", tag="st")
        nc.vector.memset(st[0:3, :PD], 1.0)
        nc.vector.tensor_copy(rposT[:], st[0:3, :PD])  # row 2 stays 1.0 forever
        # combined [u; v] per-scene blocks: tile q holds scenes (2q, 2q+1) at
        # rows {0:24 u, 32:56 v, 64:88 u, 96:120 v}; pad rows zeroed once
        # (their JI selector rows are zero, but they must be finite)
        uv = [spool.tile([PD, 512], F32R, name=f"uv{q}") for q in range(2)]
        st = stage.tile([128, 1024], F32, name="st", tag="st")
        nc.vector.memset(st[:, 0:512], 0.0)
        for q in range(2):
            nc.vector.tensor_copy(uv[q][:], st[:, 0:512])
        # full-width relu'd layer-1 tiles (one per contraction chunk):
        # column c = s*576 + h*288 + i_local*24 + j
        ybig = [spool.tile([128, 4 * 576], F32R, name=f"ybig{k}") for k in range(4)]
        # persistent pool output tiles; pad columns zeroed once
        poolT = [spool.tile([128, PD], F32R, name=f"poolT{m}") for m in range(8)]
        stz = stage.tile([128, 1024], F32, name="stz", tag="st")
        nc.vector.memset(stz[:, :PD], 0.0)
        for m in range(8):
            nc.vector.tensor_copy(poolT[m][:], stz[:, :PD])

        f32 = lambda ap: ap.bitcast(F32)

        for t in range(steps):
            # ================= LSTM =================
            gates = ps1.tile([PD, 4 * H], F32, tag="p1", name="gates")
            nc.tensor.matmul(gates[:], decT[:], wih[:], start=True, stop=False)
            nc.tensor.matmul(gates[:], hT[:], whh[:], start=False, stop=True)
            i_sig = wpool.tile([PD, H], F32, name="i_sig")
            f_sig = wpool.tile([PD, H], F32, name="f_sig")
            g_tan = wpool.tile([PD, H], F32, name="g_tan")
            o_sig = wpool.tile([PD, H], F32, name="o_sig")
            nc.scalar.activation(i_sig[:], gates[:, 0:H], AF.Sigmoid)
            nc.scalar.activation(f_sig[:], gates[:, H:2 * H], AF.Sigmoid)
            nc.scalar.activation(g_tan[:], gates[:, 2 * H:3 * H], AF.Tanh)
            nc.scalar.activation(o_sig[:], gates[:, 3 * H:4 * H], AF.Sigmoid)
            t1 = wpool.tile([PD, H], F32, name="t1")
            nc.vector.tensor_tensor(t1[:], f_sig[:], cst[:], op=OP.mult)
            t2 = wpool.tile([PD, H], F32, name="t2")
            nc.vector.tensor_tensor(t2[:], i_sig[:], g_tan[:], op=OP.mult)
            nc.vector.tensor_tensor(cst[:], t1[:], t2[:], op=OP.add)
            tanh_c = wpool.tile([PD, H], F32, name="tanh_c")
            nc.scalar.activation(tanh_c[:], cst[:], AF.Tanh)
            h_new = wpool.tile([PD, H], F32, name="h_new")
            nc.vector.tensor_tensor(h_new[:], o_sig[:], tanh_c[:], op=OP.mult)
            # h_newT via PE transpose (fp32), rounded to f32r on the ACT copy
            tr0 = ps1.tile([PD, PD], F32, tag="p1", name="tr0")
            nc.tensor.transpose(tr0[:], h_new[:], ident[:])
            h_newT = wpool.tile([H, PD], F32R, name="h_newT")
            nc.scalar.copy(h_newT[:], tr0[:, :])

            # ================= positions =================
            rp_ps = ps1.tile([2, PD], F32, tag="p1", name="rp_ps")
            nc.tensor.matmul(rp_ps[:], whp[:], f32(h_newT[:]), start=True, stop=True)
            nc.scalar.activation(rposT[0:2, :], rp_ps[:], AF.Identity, bias=bhp[:])
            dma(predT_d[t], rposT[0:2, :])
            nc.vector.tensor_tensor(posT[0:2, :], f32(posT[0:2, :]),
                                    rposT[0:2, :], op=OP.add)
            dc_ps = ps1.tile([E, PD], F32, tag="p1", name="dc_ps")
            nc.tensor.matmul(dc_ps[:], wse[:], rposT[:], start=True, stop=True)
            nc.scalar.copy(decT[0:E, :], dc_ps[:])

            # ================= u, v =================
            u_ps = ps1.tile([PD, 512], F32, tag="p1", name="u_ps")
            nc.tensor.matmul(u_ps[:], posT[:], aaug[:], start=True, stop=False)
            nc.tensor.matmul(u_ps[:], h_newT[:], wh1[:], start=False, stop=True)
            v_ps = ps1.tile([PD, 512], F32, tag="p1", name="v_ps")
            nc.tensor.matmul(v_ps[:], posT[0:2, :], aaug[0:2, :],
                             start=True, stop=True)
            # scatter u/v scene blocks into the combined uv tiles
            for s in range(SC):
                q, j = divmod(s, 2)
                nc.scalar.copy(uv[q][64 * j:64 * j + P, :],
                               u_ps[32 * s:32 * s + P, :])
                nc.scalar.copy(uv[q][64 * j + 32:64 * j + 32 + P, :],
                               v_ps[32 * s:32 * s + P, :])

            # ================= social pooling =================
            # layer-1 grid: one K=56 matmul per (k-chunk, scene, half)
            for s in range(SC):
                q, j = divmod(s, 2)
                for hh in range(2):
                    for k in range(4):
                        y_ps = ps1.tile([128, HP], F32, tag="p1", name="y_ps")
                        nc.tensor.matmul(
                            y_ps[:],
                            uv[q][64 * j:64 * j + 56, k * 128:(k + 1) * 128],
                            jimat[64 * j:64 * j + 56, hh * HP:(hh + 1) * HP],
                            start=True, stop=True, tile_position=(64 * j, 0))
                        nc.scalar.activation(
                            ybig[k][:, s * 576 + hh * HP:s * 576 + (hh + 1) * HP],
                            y_ps[:], AF.Relu)
            # layer-2 + grouped max over j: pair columns in 504-wide windows
            praw = [wpool.tile([128, 96], F32, name=f"praw{m}") for m in range(8)]
            NW = [(0, 504), (504, 504), (1008, 504), (1512, 504), (2016, 288)]
            for m in range(8):
                for w, (c0, cw) in enumerate(NW):
                    l2 = psB.tile([128, 504], F32, tag="l2", name="l2")
                    for k in range(4):
                        nc.tensor.matmul(
                            l2[:, 0:cw],
                            wp2[:, k * BNK + m * 128:k * BNK + (m + 1) * 128],
                            ybig[k][:, c0:c0 + cw],
                            start=(k == 0), stop=(k == 3))
                    nc.vector.tensor_reduce(
                        praw[m][:, c0 // P:(c0 + cw) // P],
                        l2[:, 0:cw].rearrange("p (i j) -> p i j", j=P),
                        axis=mybir.AxisListType.X, op=OP.max)
            for m in range(8):
                nc.vector.tensor_scalar(
                    poolT[m][:].rearrange("p (b c) -> p b c", c=32)[:, :, 0:P],
                    praw[m][:].rearrange("p (b c) -> p b c", c=P),
                    bp2[:, m:m + 1], 0.0, op0=OP.add, op1=OP.max)

            # ================= decoder MLP =================
            out1 = wpool.tile([PD, MLP], F32, name="out1")
            for n in range(2):
                d1 = ps1.tile([PD, 512], F32, tag="p1", name="d1")
                nc.tensor.matmul(d1[:], h_newT[:],
                                 wm1[:, n * 512:n * 512 + 512],
                                 start=True, stop=False)
                for k in range(8):
                    nc.tensor.matmul(
                        d1[:], poolT[k][:],
                        wm1[:, (k + 1) * MLP + n * 512:(k + 1) * MLP + n * 512 + 512],
                        start=False, stop=False)
                nc.tensor.matmul(d1[:], ones[:], wm1b[:, n * 512:(n + 1) * 512],
                                 start=False, stop=True)
                nc.scalar.activation(out1[:, n * 512:(n + 1) * 512], d1[:], AF.Relu)
            out1T = []
            for k in range(8):
                trk = ps1.tile([PD, PD], F32, tag="p1", name="trk")
                nc.tensor.transpose(trk[:], out1[:, k * 128:(k + 1) * 128], ident[:])
                o1t = ypool.tile([128, PD], F32, tag="ysb", name="o1t")
                nc.scalar.copy(o1t[:], trk[:])
                out1T.append(o1t)
            d2 = ps1.tile([PD, H], F32, tag="p1", name="d2")
            for k in range(8):
                nc.tensor.matmul(d2[:], out1T[k][:], wm2[:, k * H:(k + 1) * H],
                                 start=(k == 0), stop=False)
            nc.tensor.matmul(d2[:], f32(ones[:]), wm2b[:], start=False, stop=True)
            hfin = wpool.tile([PD, H], F32, name="hfin")
            nc.scalar.activation(hfin[:], d2[:], AF.Relu)
            trh = ps1.tile([PD, PD], F32, tag="p1", name="trh")
            nc.tensor.transpose(trh[:], hfin[:], ident[:])
            nc.scalar.copy(hT[:], trh[:, :])

        dma(hfinT_d[:], f32(hT[:]))

    nc.compile()
    return nc


_PROGRAM_CACHE = {}


def _get_program(steps=T):
    if steps not in _PROGRAM_CACHE:
        _PROGRAM_CACHE[steps] = build_program(steps)
    return _PROGRAM_CACHE[steps]


def _host_prep(inputs):
    """Host-side weight precompute + per-core sharding. Returns in_maps."""
    f = lambda k: np.asarray(inputs[k], np.float32)
    W_se, b_se = f("W_se"), f("b_se")
    W_ih, b_ih, W_hh, b_hh = f("W_ih"), f("b_ih"), f("W_hh"), f("b_hh")
    W_hp, b_hp = f("W_hp"), f("b_hp")
    W_pse, b_pse = f("W_pse"), f("b_pse")
    W_p1, b_p1, W_p2, b_p2 = f("W_p1"), f("b_p1"), f("W_p2"), f("b_p2")
    W_m1, b_m1, W_m2, b_m2 = f("W_m1"), f("b_m1"), f("W_m2"), f("b_m2")
    last_pos, last_pos_rel = f("last_pos"), f("last_pos_rel")
    h0, c0 = f("h0"), f("c0")

    A = W_pse @ W_p1[:E]
    b1 = b_pse @ W_p1[:E] + b_p1
    shared = {
        "wih": np.concatenate([W_ih, (b_ih + b_hh)[None]], 0),
        "whh": W_hh,
        "whp": W_hp,
        "bhp": b_hp[:, None].copy(),
        "wse": np.concatenate([W_se, b_se[None]], 0),
        "aaug": np.concatenate([A, b1[None]], 0),
        "wh1": W_p1[E:].copy(),
        "wp2": W_p2,
        "bp2": b_p2[:, None].copy(),
        "wm1": np.concatenate([W_m1, b_m1[None]], 0),
        "wm2": np.concatenate([W_m2, b_m2[None]], 0),
        "ident": np.eye(PD, dtype=np.float32),
    }
    # combined selector matrix: per 64-row block, rows 0:24 pick u_j, rows
    # 32:56 subtract v_i; column c = h*288 + i_local*24 + j
    JI = np.zeros((64, 2 * HP), np.float32)
    for h in range(2):
        for i_l in range(12):
            for jj in range(P):
                JI[jj, h * HP + i_l * P + jj] = 1.0
                JI[32 + 12 * h + i_l, h * HP + i_l * P + jj] = -1.0
    shared["jimat"] = np.concatenate([JI, JI], 0)

    dec0 = last_pos_rel @ W_se + b_se    # (B, E)

    def pad_pedsT(x):  # (n_peds_core, D) -> (D, PD) scene-blocked
        D = x.shape[1]
        out = np.zeros((D, PD), np.float32)
        for s in range(SC):
            out[:, 32 * s:32 * s + P] = x[s * P:(s + 1) * P].T
        return out

    in_maps = []
    for core in range(N_CORES):
        sl = slice(core * SC * P, (core + 1) * SC * P)
        m = dict(shared)
        m["h0T"] = pad_pedsT(h0[0, sl])
        m["c0p"] = pad_pedsT(c0[0, sl]).T.copy()
        m["lposT"] = pad_pedsT(last_pos[sl])[:2].copy()
        m["dec0T"] = pad_pedsT(dec0[sl])
        in_maps.append(m)
    return in_maps


def _unpack(results):
    pred = np.zeros((T, B, 2), np.float32)
    hfin = np.zeros((1, B, H), np.float32)
    for core in range(N_CORES):
        pT = results[core]["predT"]   # (T, 2, PD)
        hT = results[core]["hfinT"]   # (H, PD)
        for s in range(SC):
            gsl = slice((core * SC + s) * P, (core * SC + s + 1) * P)
            psl = slice(32 * s, 32 * s + P)
            pred[:, gsl, :] = pT[:, :, psl].transpose(0, 2, 1)
            hfin[0, gsl, :] = hT[:, psl].T
    return pred, hfin


def kernel(**inputs):
    nc = _get_program()
    in_maps = _host_prep(inputs)
    res = run_bass_kernel_spmd(nc, in_maps, list(range(N_CORES)))
    return _unpack(res.results)


# revision 21
# speedup vs baseline: 1.1675x; 1.0579x over previous
"""Trainium2 Bass kernel for the SocialGAN-style decoder (nn_Decoder).

Sharding: data-parallel over scenes. 32 scenes x 24 peds; 8 cores get 4
scenes (96 peds) each, padded to 128 partition lanes (each scene in a
32-lane block, 24 used). Weights are replicated. No cross-core comms.

Algorithmic notes (validated vs reference in fp32 numpy + CoreSim):
  pre_relu[i,j,:] = emb(pos_j - pos_i) @ W_p1[:E] + h_j @ W_p1[E:] + b
                  = u_j - v_i
  with u = pos @ A + h @ Wh1 + b1, v = pos @ A, A = W_pse @ W_p1[:E],
  b1 = b_pse @ W_p1[:E] + b_p1. The (i,j) broadcast grid is emitted as
  PE matmuls against constant selector matrices J (pick j) and -I
  (pick i), so layer 1 of the pool MLP never touches the vector engine.
  relu/max commute: pool = relu(max_j(l2) + b_p2).

Big matmuls run in float32r (full-rate PE, ~2e-4 relative rounding).
fp32r operands must be produced by compute engines (ACT/DVE round on
write); DMA-loaded weights go through a one-time DVE rounding copy.
"""
import numpy as np

import concourse.bacc as bacc
import concourse.mybir as mybir
import concourse.tile as tile
from concourse.bass_utils import run_bass_kernel_spmd

F32 = mybir.dt.float32
F32R = mybir.dt.float32r

# problem dims (hardcoded per contract)
T = 12
E, H, BNK, MLP = 64, 128, 1024, 1024
S, P = 32, 24
B = S * P
N_CORES = 8
SC = S // N_CORES          # scenes per core = 4
PD = 128                   # padded peds per core (4 x 32-lane blocks)
HP = P * P // 2            # pair-columns per half-scene = 288


def build_program(steps=T):
    nc = bacc.Bacc("TRN2", target_bir_lowering=False, debug=False)

    def din(name, shape):
        return nc.dram_tensor(name, list(shape), F32, kind="ExternalInput").ap()

    def dout(name, shape):
        return nc.dram_tensor(name, list(shape), F32, kind="ExternalOutput").ap()

    # per-core sharded state
    h0T_d = din("h0T", (H, PD))
    c0_d = din("c0p", (PD, H))
    lposT_d = din("lposT", (2, PD))
    dec0T_d = din("dec0T", (E, PD))
    # replicated (pre-combined) weights
    wih_d = din("wih", (E + 1, 4 * H))      # [W_ih; b_ih+b_hh]
    whh_d = din("whh", (H, 4 * H))
    whp_d = din("whp", (H, 2))
    bhp_d = din("bhp", (2, 1))
    wse_d = din("wse", (3, E))              # [W_se; b_se]
    aaug_d = din("aaug", (3, 512))          # [A; b1]
    wh1_d = din("wh1", (H, 512))            # W_p1[E:]
    jimat_d = din("jimat", (PD, 2 * HP))    # [J; 0; -I] per 64-block
    wp2_d = din("wp2", (512, BNK))
    bp2_d = din("bp2", (BNK, 1))
    wm1_d = din("wm1", (H + BNK + 1, MLP))  # [W_m1; b_m1]
    wm2_d = din("wm2", (BNK + 1, H))        # [W_m2; b_m2]
    ident_d = din("ident", (PD, PD))
    predT_d = dout("predT", (T, 2, PD))
    hfinT_d = dout("hfinT", (H, PD))

    AF = mybir.ActivationFunctionType
    OP = mybir.AluOpType

    from contextlib import ExitStack
    with tile.TileContext(nc) as tc, ExitStack() as ctx:
        cpool = ctx.enter_context(tc.tile_pool(name="consts", bufs=1))
        spool = ctx.enter_context(tc.tile_pool(name="state", bufs=1))
        stage = ctx.enter_context(tc.tile_pool(name="stage", bufs=2))
        wpool = ctx.enter_context(tc.tile_pool(name="work", bufs=2))
        ypool = ctx.enter_context(tc.tile_pool(name="ysb", bufs=16))
        ps1 = ctx.enter_context(tc.tile_pool(name="ps1", bufs=4, space="PSUM"))
        psB = ctx.enter_context(tc.tile_pool(name="psB", bufs=4, space="PSUM"))

        dma = nc.sync.dma_start

        # ---- load fp32 constants (direct) ----
        def cload(name, ap, dt=F32):
            t = cpool.tile(list(ap.shape), dt, name=name)
            dma(t[:], ap[:])
            return t

        whp = cload("whp", whp_d)
        bhp = cload("bhp", bhp_d)
        wse = cload("wse", wse_d)
        ident = cload("ident", ident_d)
        bp2 = cpool.tile([128, 8], F32, name="bp2")
        dma(bp2[:], bp2_d.rearrange("(m p) one -> p (m one)", p=128))
        wm2 = cpool.tile([128, 8 * H], F32, name="wm2")
        for k in range(8):
            dma(wm2[:, k * H:(k + 1) * H], wm2_d[k * 128:(k + 1) * 128, :])
        wm2b = cpool.tile([1, H], F32, name="wm2b")
        dma(wm2b[:], wm2_d[BNK:BNK + 1, :])

        # ---- fp32r constants: DMA to staging, DVE rounding-copy in ----
        def rload(name, src_ap, chunks=1):
            """Load a DRAM tensor into an F32R tile via staging copies.
            src_ap is (rows<=128, cols); chunks splits cols for staging."""
            rows, cols = src_ap.shape
            t = cpool.tile([rows, cols], F32R, name=name)
            cc = cols // chunks
            for i in range(chunks):
                st = stage.tile([128, 1024], F32, name="st", tag="st")
                dma(st[:rows, :cc], src_ap[:, i * cc:(i + 1) * cc])
                nc.vector.tensor_copy(t[:, i * cc:(i + 1) * cc], st[:rows, :cc])
            return t

        wih = rload("wih", wih_d)
        whh = rload("whh", whh_d)
        aaug = rload("aaug", aaug_d)
        wh1 = rload("wh1", wh1_d)
        jimat = rload("jimat", jimat_d)
        wp2 = cpool.tile([128, 4 * BNK], F32R, name="wp2")  # [k*1024 + m*128]
        for k in range(4):
            st = stage.tile([128, 1024], F32, name="st", tag="st")
            dma(st[:], wp2_d[k * 128:(k + 1) * 128, :])
            nc.vector.tensor_copy(wp2[:, k * BNK:(k + 1) * BNK], st[:])
        wm1 = cpool.tile([128, 9 * MLP], F32R, name="wm1")
        for k in range(9):
            st = stage.tile([128, 1024], F32, name="st", tag="st")
            dma(st[:], wm1_d[k * 128:(k + 1) * 128, :])
            nc.vector.tensor_copy(wm1[:, k * MLP:(k + 1) * MLP], st[:])
        wm1b = cpool.tile([1, MLP], F32R, name="wm1b")
        st = stage.tile([128, 1024], F32, name="st", tag="st")
        dma(st[0:1, :], wm1_d[H + BNK:H + BNK + 1, :])
        nc.vector.tensor_copy(wm1b[:], st[0:1, :])
        ones = cpool.tile([1, PD], F32R, name="ones")
        st = stage.tile([128, 1024], F32, name="st", tag="st")
        nc.vector.memset(st[0:1, :PD], 1.0)
        nc.vector.tensor_copy(ones[:], st[0:1, :PD])

        # ---- state (fp32r where matmul-consumed) ----
        hT = spool.tile([H, PD], F32R, name="hT")         # carry h (transposed)
        st = stage.tile([128, 1024], F32, name="st", tag="st")
        dma(st[:, :PD], h0T_d[:])
        nc.vector.tensor_copy(hT[:], st[:, :PD])
        cst = spool.tile([PD, H], F32, name="cst")        # carry c
        dma(cst[:], c0_d[:])
        posT = spool.tile([3, PD], F32R, name="posT")     # rows 0:2 pos, row 2 ones
        st = stage.tile([128, 1024], F32, name="st", tag="st")
        nc.vector.memset(st[0:3, :PD], 1.0)
        dma(st[0:2, :PD], lposT_d[:])
        nc.vector.tensor_copy(posT[:], st[0:3, :PD])
        decT = spool.tile([E + 1, PD], F32R, name="decT")  # rows 0:E dec, row E ones
        st = stage.tile([128, 1024], F32, name="st", tag="st")
        dma(st[0:E, :PD], dec0T_d[:])
        nc.vector.memset(st[E:E + 1, :PD], 1.0)
        nc.vector.tensor_copy(decT[:], st[0:E + 1, :PD])
        rposT = spool.tile([3, PD], F32, name="rposT")    # rows 0:2 rel, row 2 ones
        st = stage.tile([128, 1024], F32, name="st", tag="st")
        nc.vector.memset(st[0:3, :PD], 1.0)
        nc.vector.tensor_copy(rposT[:], st[0:3, :PD])  # row 2 stays 1.0 forever
        # combined [u; v] per-scene blocks: tile q holds scenes (2q, 2q+1) at
        # rows {0:24 u, 32:56 v, 64:88 u, 96:120 v}; pad rows zeroed once
        # (their JI selector rows are zero, but they must be finite)
        uv = [spool.tile([PD, 512], F32R, name=f"uv{q}") for q in range(2)]
        st = stage.tile([128, 1024], F32, name="st", tag="st")
        nc.vector.memset(st[:, 0:512], 0.0)
        for q in range(2):
            nc.vector.tensor_copy(uv[q][:], st[:, 0:512])
        # full-width relu'd layer-1 tiles (one per contraction chunk):
        # column c = s*576 + h*288 + i_local*24 + j
        ybig = [spool.tile([128, 4 * 576], F32R, name=f"ybig{k}") for k in range(4)]
        # persistent pool output tiles; pad columns zeroed once
        poolT = [spool.tile([128, PD], F32R, name=f"poolT{m}") for m in range(8)]
        stz = stage.tile([128, 1024], F32, name="stz", tag="st")
        nc.vector.memset(stz[:, :PD], 0.0)
        for m in range(8):
            nc.vector.tensor_copy(poolT[m][:], stz[:, :PD])

        f32 = lambda ap: ap.bitcast(F32)

        for t in range(steps):
            # ================= LSTM =================
            gates = ps1.tile([PD, 4 * H], F32, tag="p1", name="gates")
            nc.tensor.matmul(gates[:], decT[:], wih[:], start=True, stop=False)
            nc.tensor.matmul(gates[:], hT[:], whh[:], start=False, stop=True)
            i_sig = wpool.tile([PD, H], F32, name="i_sig")
            f_sig = wpool.tile([PD, H], F32, name="f_sig")
            g_tan = wpool.tile([PD, H], F32, name="g_tan")
            o_sig = wpool.tile([PD, H], F32, name="o_sig")
            nc.scalar.activation(i_sig[:], gates[:, 0:H], AF.Sigmoid)
            nc.scalar.activation(f_sig[:], gates[:, H:2 * H], AF.Sigmoid)
            nc.scalar.activation(g_tan[:], gates[:, 2 * H:3 * H], AF.Tanh)
            nc.scalar.activation(o_sig[:], gates[:, 3 * H:4 * H], AF.Sigmoid)
            t1 = wpool.tile([PD, H], F32, name="t1")
            nc.vector.tensor_tensor(t1[:], f_sig[:], cst[:], op=OP.mult)
            t2 = wpool.tile([PD, H], F32, name="t2")
            nc.vector.tensor_tensor(t2[:], i_sig[:], g_tan[:], op=OP.mult)
            nc.vector.tensor_tensor(cst[:], t1[:], t2[:], op=OP.add)
            tanh_c = wpool.tile([PD, H], F32, name="tanh_c")
            nc.scalar.activation(tanh_c[:], cst[:], AF.Tanh)
            h_new = wpool.tile([PD, H], F32, name="h_new")
            nc.vector.tensor_tensor(h_new[:], o_sig[:], tanh_c[:], op=OP.mult)
            # h_newT via PE transpose (fp32), rounded to f32r on the ACT copy
            tr0 = ps1.tile([PD, PD], F32, tag="p1", name="tr0")
            nc.tensor.transpose(tr0[:], h_new[:], ident[:])
            h_newT = wpool.tile([H, PD], F32R, name="h_newT")
            nc.scalar.copy(h_newT[:], tr0[:, :])

            # ================= positions =================
            rp_ps = ps1.tile([2, PD], F32, tag="p1", name="rp_ps")
            nc.tensor.matmul(rp_ps[:], whp[:], f32(h_newT[:]), start=True, stop=True)
            nc.scalar.activation(rposT[0:2, :], rp_ps[:], AF.Identity, bias=bhp[:])
            dma(predT_d[t], rposT[0:2, :])
            nc.vector.tensor_tensor(posT[0:2, :], f32(posT[0:2, :]),
                                    rposT[0:2, :], op=OP.add)
            dc_ps = ps1.tile([E, PD], F32, tag="p1", name="dc_ps")
            nc.tensor.matmul(dc_ps[:], wse[:], rposT[:], start=True, stop=True)
            nc.scalar.copy(decT[0:E, :], dc_ps[:])

            # ================= u, v =================
            u_ps = ps1.tile([PD, 512], F32, tag="p1", name="u_ps")
            nc.tensor.matmul(u_ps[:], posT[:], aaug[:], start=True, stop=False)
            nc.tensor.matmul(u_ps[:], h_newT[:], wh1[:], start=False, stop=True)
            v_ps = ps1.tile([PD, 512], F32, tag="p1", name="v_ps")
            nc.tensor.matmul(v_ps[:], posT[0:2, :], aaug[0:2, :],
                             start=True, stop=True)
            # scatter u/v scene blocks into the combined uv tiles
            for s in range(SC):
                q, j = divmod(s, 2)
                nc.scalar.copy(uv[q][64 * j:64 * j + P, :],
                               u_ps[32 * s:32 * s + P, :])
                nc.scalar.copy(uv[q][64 * j + 32:64 * j + 32 + P, :],
                               v_ps[32 * s:32 * s + P, :])

            # ================= social pooling =================
            # layer-1 grid (one K=56 matmul per chunk/scene/half) interleaved
            # with layer-2 windows: after scene s's relus land, the 504-wide
            # windows that only touch scenes <= s are emitted, so the PE
            # stays dense while the next scene's relu chain runs on ACT.
            praw = [wpool.tile([128, 96], F32, name=f"praw{m}") for m in range(8)]
            NW = [(0, 504), (504, 504), (1008, 504), (1512, 504), (2016, 288)]
            ready_w = {0: [0], 1: [1], 2: [2], 3: [3, 4]}  # scene -> windows

            def emit_y1(s, hh, k):
                q, j = divmod(s, 2)
                y_ps = ps1.tile([128, HP], F32, tag="p1", name="y_ps")
                nc.tensor.matmul(
                    y_ps[:],
                    uv[q][64 * j:64 * j + 56, k * 128:(k + 1) * 128],
                    jimat[64 * j:64 * j + 56, hh * HP:(hh + 1) * HP],
                    start=True, stop=True, tile_position=(64 * j, 0))
                nc.scalar.activation(
                    ybig[k][:, s * 576 + hh * HP:s * 576 + (hh + 1) * HP],
                    y_ps[:], AF.Relu)

            def emit_l2m(w, m):
                c0, cw = NW[w]
                l2 = psB.tile([128, 504], F32, tag="l2", name="l2")
                for k in range(4):
                    nc.tensor.matmul(
                        l2[:, 0:cw],
                        wp2[:, k * BNK + m * 128:k * BNK + (m + 1) * 128],
                        ybig[k][:, c0:c0 + cw],
                        start=(k == 0), stop=(k == 3))
                nc.vector.tensor_reduce(
                    praw[m][:, c0 // P:(c0 + cw) // P],
                    l2[:, 0:cw].rearrange("p (i j) -> p i j", j=P),
                    axis=mybir.AxisListType.X, op=OP.max)

            for hh in range(2):
                for k in range(4):
                    emit_y1(0, hh, k)
            for s in range(1, SC):
                w = ready_w[s - 1][0]
                for i, (hh, k) in enumerate([(a, b) for a in range(2)
                                             for b in range(4)]):
                    emit_y1(s, hh, k)
                    emit_l2m(w, i)
            for w in ready_w[SC - 1]:
                for m in range(8):
                    emit_l2m(w, m)
            for m in range(8):
                nc.vector.tensor_scalar(
                    poolT[m][:].rearrange("p (b c) -> p b c", c=32)[:, :, 0:P],
                    praw[m][:].rearrange("p (b c) -> p b c", c=P),
                    bp2[:, m:m + 1], 0.0, op0=OP.add, op1=OP.max)

            # ================= decoder MLP =================
            out1 = wpool.tile([PD, MLP], F32, name="out1")
            for n in range(2):
                d1 = ps1.tile([PD, 512], F32, tag="p1", name="d1")
                nc.tensor.matmul(d1[:], h_newT[:],
                                 wm1[:, n * 512:n * 512 + 512],
                                 start=True, stop=False)
                for k in range(8):
                    nc.tensor.matmul(
                        d1[:], poolT[k][:],
                        wm1[:, (k + 1) * MLP + n * 512:(k + 1) * MLP + n * 512 + 512],
                        start=False, stop=False)
                nc.tensor.matmul(d1[:], ones[:], wm1b[:, n * 512:(n + 1) * 512],
                                 start=False, stop=True)
                nc.scalar.activation(out1[:, n * 512:(n + 1) * 512], d1[:], AF.Relu)
            out1T = []
            for k in range(8):
                trk = ps1.tile([PD, PD], F32, tag="p1", name="trk")
                nc.tensor.transpose(trk[:], out1[:, k * 128:(k + 1) * 128], ident[:])
                o1t = ypool.tile([128, PD], F32, tag="ysb", name="o1t")
                nc.scalar.copy(o1t[:], trk[:])
                out1T.append(o1t)
            d2 = ps1.tile([PD, H], F32, tag="p1", name="d2")
            for k in range(8):
                nc.tensor.matmul(d2[:], out1T[k][:], wm2[:, k * H:(k + 1) * H],
                                 start=(k == 0), stop=False)
            nc.tensor.matmul(d2[:], f32(ones[:]), wm2b[:], start=False, stop=True)
            hfin = wpool.tile([PD, H], F32, name="hfin")
            nc.scalar.activation(hfin[:], d2[:], AF.Relu)
            trh = ps1.tile([PD, PD], F32, tag="p1", name="trh")
            nc.tensor.transpose(trh[:], hfin[:], ident[:])
            nc.scalar.copy(hT[:], trh[:, :])

        dma(hfinT_d[:], f32(hT[:]))

    nc.compile()
    return nc


_PROGRAM_CACHE = {}


def _get_program(steps=T):
    if steps not in _PROGRAM_CACHE:
        _PROGRAM_CACHE[steps] = build_program(steps)
    return _PROGRAM_CACHE[steps]


def _host_prep(inputs):
    """Host-side weight precompute + per-core sharding. Returns in_maps."""
    f = lambda k: np.asarray(inputs[k], np.float32)
    W_se, b_se = f("W_se"), f("b_se")
    W_ih, b_ih, W_hh, b_hh = f("W_ih"), f("b_ih"), f("W_hh"), f("b_hh")
    W_hp, b_hp = f("W_hp"), f("b_hp")
    W_pse, b_pse = f("W_pse"), f("b_pse")
    W_p1, b_p1, W_p2, b_p2 = f("W_p1"), f("b_p1"), f("W_p2"), f("b_p2")
    W_m1, b_m1, W_m2, b_m2 = f("W_m1"), f("b_m1"), f("W_m2"), f("b_m2")
    last_pos, last_pos_rel = f("last_pos"), f("last_pos_rel")
    h0, c0 = f("h0"), f("c0")

    A = W_pse @ W_p1[:E]
    b1 = b_pse @ W_p1[:E] + b_p1
    shared = {
        "wih": np.concatenate([W_ih, (b_ih + b_hh)[None]], 0),
        "whh": W_hh,
        "whp": W_hp,
        "bhp": b_hp[:, None].copy(),
        "wse": np.concatenate([W_se, b_se[None]], 0),
        "aaug": np.concatenate([A, b1[None]], 0),
        "wh1": W_p1[E:].copy(),
        "wp2": W_p2,
        "bp2": b_p2[:, None].copy(),
        "wm1": np.concatenate([W_m1, b_m1[None]], 0),
        "wm2": np.concatenate([W_m2, b_m2[None]], 0),
        "ident": np.eye(PD, dtype=np.float32),
    }
    # combined selector matrix: per 64-row block, rows 0:24 pick u_j, rows
    # 32:56 subtract v_i; column c = h*288 + i_local*24 + j
    JI = np.zeros((64, 2 * HP), np.float32)
    for h in range(2):
        for i_l in range(12):
            for jj in range(P):
                JI[jj, h * HP + i_l * P + jj] = 1.0
                JI[32 + 12 * h + i_l, h * HP + i_l * P + jj] = -1.0
    shared["jimat"] = np.concatenate([JI, JI], 0)

    dec0 = last_pos_rel @ W_se + b_se    # (B, E)

    def pad_pedsT(x):  # (n_peds_core, D) -> (D, PD) scene-blocked
        D = x.shape[1]
        out = np.zeros((D, PD), np.float32)
        for s in range(SC):
            out[:, 32 * s:32 * s + P] = x[s * P:(s + 1) * P].T
        return out

    in_maps = []
    for core in range(N_CORES):
        sl = slice(core * SC * P, (core + 1) * SC * P)
        m = dict(shared)
        m["h0T"] = pad_pedsT(h0[0, sl])
        m["c0p"] = pad_pedsT(c0[0, sl]).T.copy()
        m["lposT"] = pad_pedsT(last_pos[sl])[:2].copy()
        m["dec0T"] = pad_pedsT(dec0[sl])
        in_maps.append(m)
    return in_maps


def _unpack(results):
    pred = np.zeros((T, B, 2), np.float32)
    hfin = np.zeros((1, B, H), np.float32)
    for core in range(N_CORES):
        pT = results[core]["predT"]   # (T, 2, PD)
        hT = results[core]["hfinT"]   # (H, PD)
        for s in range(SC):
            gsl = slice((core * SC + s) * P, (core * SC + s + 1) * P)
            psl = slice(32 * s, 32 * s + P)
            pred[:, gsl, :] = pT[:, :, psl].transpose(0, 2, 1)
            hfin[0, gsl, :] = hT[:, psl].T
    return pred, hfin


def kernel(**inputs):
    nc = _get_program()
    in_maps = _host_prep(inputs)
    res = run_bass_kernel_spmd(nc, in_maps, list(range(N_CORES)))
    return _unpack(res.results)


# revision 22
# speedup vs baseline: 1.1725x; 1.0043x over previous
"""Trainium2 Bass kernel for the SocialGAN-style decoder (nn_Decoder).

Sharding: data-parallel over scenes. 32 scenes x 24 peds; 8 cores get 4
scenes (96 peds) each, padded to 128 partition lanes (each scene in a
32-lane block, 24 used). Weights are replicated. No cross-core comms.

Algorithmic notes (validated vs reference in fp32 numpy + CoreSim):
  pre_relu[i,j,:] = emb(pos_j - pos_i) @ W_p1[:E] + h_j @ W_p1[E:] + b
                  = u_j - v_i
  with u = pos @ A + h @ Wh1 + b1, v = pos @ A, A = W_pse @ W_p1[:E],
  b1 = b_pse @ W_p1[:E] + b_p1. The (i,j) broadcast grid is emitted as
  PE matmuls against constant selector matrices J (pick j) and -I
  (pick i), so layer 1 of the pool MLP never touches the vector engine.
  relu/max commute: pool = relu(max_j(l2) + b_p2).

Big matmuls run in float32r (full-rate PE, ~2e-4 relative rounding).
fp32r operands must be produced by compute engines (ACT/DVE round on
write); DMA-loaded weights go through a one-time DVE rounding copy.
"""
import numpy as np

import concourse.bacc as bacc
import concourse.mybir as mybir
import concourse.tile as tile
from concourse.bass_utils import run_bass_kernel_spmd

F32 = mybir.dt.float32
F32R = mybir.dt.float32r

# problem dims (hardcoded per contract)
T = 12
E, H, BNK, MLP = 64, 128, 1024, 1024
S, P = 32, 24
B = S * P
N_CORES = 8
SC = S // N_CORES          # scenes per core = 4
PD = 128                   # padded peds per core (4 x 32-lane blocks)
HP = P * P // 2            # pair-columns per half-scene = 288


def build_program(steps=T):
    nc = bacc.Bacc("TRN2", target_bir_lowering=False, debug=False)

    def din(name, shape):
        return nc.dram_tensor(name, list(shape), F32, kind="ExternalInput").ap()

    def dout(name, shape):
        return nc.dram_tensor(name, list(shape), F32, kind="ExternalOutput").ap()

    # per-core sharded state
    h0T_d = din("h0T", (H, PD))
    c0_d = din("c0p", (PD, H))
    lposT_d = din("lposT", (2, PD))
    dec0T_d = din("dec0T", (E, PD))
    # replicated (pre-combined) weights
    wih_d = din("wih", (E + 1, 4 * H))      # [W_ih; b_ih+b_hh]
    whh_d = din("whh", (H, 4 * H))
    whp_d = din("whp", (H, 2))
    bhp_d = din("bhp", (2, 1))
    wse_d = din("wse", (3, E))              # [W_se; b_se]
    aaug_d = din("aaug", (3, 512))          # [A; b1]
    wh1_d = din("wh1", (H, 512))            # W_p1[E:]
    jimat_d = din("jimat", (PD, 2 * HP))    # [J; 0; -I] per 64-block
    wp2_d = din("wp2", (512, BNK))
    bp2_d = din("bp2", (BNK, 1))
    wm1_d = din("wm1", (H + BNK + 1, MLP))  # [W_m1; b_m1]
    wm2_d = din("wm2", (BNK + 1, H))        # [W_m2; b_m2]
    ident_d = din("ident", (PD, PD))
    predT_d = dout("predT", (T, 2, PD))
    hfinT_d = dout("hfinT", (H, PD))

    AF = mybir.ActivationFunctionType
    OP = mybir.AluOpType

    from contextlib import ExitStack
    with tile.TileContext(nc) as tc, ExitStack() as ctx:
        cpool = ctx.enter_context(tc.tile_pool(name="consts", bufs=1))
        spool = ctx.enter_context(tc.tile_pool(name="state", bufs=1))
        stage = ctx.enter_context(tc.tile_pool(name="stage", bufs=2))
        wpool = ctx.enter_context(tc.tile_pool(name="work", bufs=2))
        ypool = ctx.enter_context(tc.tile_pool(name="ysb", bufs=16))
        ps1 = ctx.enter_context(tc.tile_pool(name="ps1", bufs=3, space="PSUM"))
        psB = ctx.enter_context(tc.tile_pool(name="psB", bufs=5, space="PSUM"))

        dma = nc.sync.dma_start

        # ---- load fp32 constants (direct) ----
        def cload(name, ap, dt=F32):
            t = cpool.tile(list(ap.shape), dt, name=name)
            dma(t[:], ap[:])
            return t

        whp = cload("whp", whp_d)
        bhp = cload("bhp", bhp_d)
        wse = cload("wse", wse_d)
        ident = cload("ident", ident_d)
        bp2 = cpool.tile([128, 8], F32, name="bp2")
        dma(bp2[:], bp2_d.rearrange("(m p) one -> p (m one)", p=128))
        wm2 = cpool.tile([128, 8 * H], F32, name="wm2")
        for k in range(8):
            dma(wm2[:, k * H:(k + 1) * H], wm2_d[k * 128:(k + 1) * 128, :])
        wm2b = cpool.tile([1, H], F32, name="wm2b")
        dma(wm2b[:], wm2_d[BNK:BNK + 1, :])

        # ---- fp32r constants: DMA to staging, DVE rounding-copy in ----
        def rload(name, src_ap, chunks=1):
            """Load a DRAM tensor into an F32R tile via staging copies.
            src_ap is (rows<=128, cols); chunks splits cols for staging."""
            rows, cols = src_ap.shape
            t = cpool.tile([rows, cols], F32R, name=name)
            cc = cols // chunks
            for i in range(chunks):
                st = stage.tile([128, 1024], F32, name="st", tag="st")
                dma(st[:rows, :cc], src_ap[:, i * cc:(i + 1) * cc])
                nc.vector.tensor_copy(t[:, i * cc:(i + 1) * cc], st[:rows, :cc])
            return t

        wih = rload("wih", wih_d)
        whh = rload("whh", whh_d)
        aaug = rload("aaug", aaug_d)
        wh1 = rload("wh1", wh1_d)
        jimat = rload("jimat", jimat_d)
        wp2 = cpool.tile([128, 4 * BNK], F32R, name="wp2")  # [k*1024 + m*128]
        for k in range(4):
            st = stage.tile([128, 1024], F32, name="st", tag="st")
            dma(st[:], wp2_d[k * 128:(k + 1) * 128, :])
            nc.vector.tensor_copy(wp2[:, k * BNK:(k + 1) * BNK], st[:])
        wm1 = cpool.tile([128, 9 * MLP], F32R, name="wm1")
        for k in range(9):
            st = stage.tile([128, 1024], F32, name="st", tag="st")
            dma(st[:], wm1_d[k * 128:(k + 1) * 128, :])
            nc.vector.tensor_copy(wm1[:, k * MLP:(k + 1) * MLP], st[:])
        wm1b = cpool.tile([1, MLP], F32R, name="wm1b")
        st = stage.tile([128, 1024], F32, name="st", tag="st")
        dma(st[0:1, :], wm1_d[H + BNK:H + BNK + 1, :])
        nc.vector.tensor_copy(wm1b[:], st[0:1, :])
        ones = cpool.tile([1, PD], F32R, name="ones")
        st = stage.tile([128, 1024], F32, name="st", tag="st")
        nc.vector.memset(st[0:1, :PD], 1.0)
        nc.vector.tensor_copy(ones[:], st[0:1, :PD])

        # ---- state (fp32r where matmul-consumed) ----
        hT = spool.tile([H, PD], F32R, name="hT")         # carry h (transposed)
        st = stage.tile([128, 1024], F32, name="st", tag="st")
        dma(st[:, :PD], h0T_d[:])
        nc.vector.tensor_copy(hT[:], st[:, :PD])
        cst = spool.tile([PD, H], F32, name="cst")        # carry c
        dma(cst[:], c0_d[:])
        posT = spool.tile([3, PD], F32R, name="posT")     # rows 0:2 pos, row 2 ones
        st = stage.tile([128, 1024], F32, name="st", tag="st")
        nc.vector.memset(st[0:3, :PD], 1.0)
        dma(st[0:2, :PD], lposT_d[:])
        nc.vector.tensor_copy(posT[:], st[0:3, :PD])
        decT = spool.tile([E + 1, PD], F32R, name="decT")  # rows 0:E dec, row E ones
        st = stage.tile([128, 1024], F32, name="st", tag="st")
        dma(st[0:E, :PD], dec0T_d[:])
        nc.vector.memset(st[E:E + 1, :PD], 1.0)
        nc.vector.tensor_copy(decT[:], st[0:E + 1, :PD])
        rposT = spool.tile([3, PD], F32, name="rposT")    # rows 0:2 rel, row 2 ones
        st = stage.tile([128, 1024], F32, name="st", tag="st")
        nc.vector.memset(st[0:3, :PD], 1.0)
        nc.vector.tensor_copy(rposT[:], st[0:3, :PD])  # row 2 stays 1.0 forever
        # combined [u; v] per-scene blocks: tile q holds scenes (2q, 2q+1) at
        # rows {0:24 u, 32:56 v, 64:88 u, 96:120 v}; pad rows zeroed once
        # (their JI selector rows are zero, but they must be finite)
        uv = [spool.tile([PD, 512], F32R, name=f"uv{q}") for q in range(2)]
        st = stage.tile([128, 1024], F32, name="st", tag="st")
        nc.vector.memset(st[:, 0:512], 0.0)
        for q in range(2):
            nc.vector.tensor_copy(uv[q][:], st[:, 0:512])
        # full-width relu'd layer-1 tiles (one per contraction chunk):
        # column c = s*576 + h*288 + i_local*24 + j
        ybig = [spool.tile([128, 4 * 576], F32R, name=f"ybig{k}") for k in range(4)]
        # persistent pool output tiles; pad columns zeroed once
        poolT = [spool.tile([128, PD], F32R, name=f"poolT{m}") for m in range(8)]
        stz = stage.tile([128, 1024], F32, name="stz", tag="st")
        nc.vector.memset(stz[:, :PD], 0.0)
        for m in range(8):
            nc.vector.tensor_copy(poolT[m][:], stz[:, :PD])

        f32 = lambda ap: ap.bitcast(F32)

        for t in range(steps):
            # ================= LSTM =================
            gates = ps1.tile([PD, 4 * H], F32, tag="p1", name="gates")
            nc.tensor.matmul(gates[:], decT[:], wih[:], start=True, stop=False)
            nc.tensor.matmul(gates[:], hT[:], whh[:], start=False, stop=True)
            if_sig = wpool.tile([PD, 2 * H], F32, name="if_sig")
            g_tan = wpool.tile([PD, H], F32, name="g_tan")
            o_sig = wpool.tile([PD, H], F32, name="o_sig")
            nc.scalar.activation(if_sig[:], gates[:, 0:2 * H], AF.Sigmoid)
            nc.scalar.activation(g_tan[:], gates[:, 2 * H:3 * H], AF.Tanh)
            nc.scalar.activation(o_sig[:], gates[:, 3 * H:4 * H], AF.Sigmoid)
            t1 = wpool.tile([PD, H], F32, name="t1")
            nc.vector.tensor_tensor(t1[:], if_sig[:, H:2 * H], cst[:], op=OP.mult)
            t2 = wpool.tile([PD, H], F32, name="t2")
            nc.vector.tensor_tensor(t2[:], if_sig[:, 0:H], g_tan[:], op=OP.mult)
            nc.vector.tensor_tensor(cst[:], t1[:], t2[:], op=OP.add)
            tanh_c = wpool.tile([PD, H], F32, name="tanh_c")
            nc.scalar.activation(tanh_c[:], cst[:], AF.Tanh)
            h_new = wpool.tile([PD, H], F32, name="h_new")
            nc.vector.tensor_tensor(h_new[:], o_sig[:], tanh_c[:], op=OP.mult)
            # h_newT via PE transpose (fp32), rounded to f32r on the ACT copy
            tr0 = ps1.tile([PD, PD], F32, tag="p1", name="tr0")
            nc.tensor.transpose(tr0[:], h_new[:], ident[:])
            h_newT = wpool.tile([H, PD], F32R, name="h_newT")
            nc.scalar.copy(h_newT[:], tr0[:, :])

            # ================= positions =================
            rp_ps = ps1.tile([2, PD], F32, tag="p1", name="rp_ps")
            nc.tensor.matmul(rp_ps[:], whp[:], f32(h_newT[:]), start=True, stop=True)
            nc.scalar.activation(rposT[0:2, :], rp_ps[:], AF.Identity, bias=bhp[:])
            dma(predT_d[t], rposT[0:2, :])
            nc.vector.tensor_tensor(posT[0:2, :], f32(posT[0:2, :]),
                                    rposT[0:2, :], op=OP.add)
            dc_ps = ps1.tile([E, PD], F32, tag="p1", name="dc_ps")
            nc.tensor.matmul(dc_ps[:], wse[:], rposT[:], start=True, stop=True)
            nc.scalar.copy(decT[0:E, :], dc_ps[:])

            # ================= u, v =================
            u_ps = ps1.tile([PD, 512], F32, tag="p1", name="u_ps")
            nc.tensor.matmul(u_ps[:], posT[:], aaug[:], start=True, stop=False)
            nc.tensor.matmul(u_ps[:], h_newT[:], wh1[:], start=False, stop=True)
            v_ps = ps1.tile([PD, 512], F32, tag="p1", name="v_ps")
            nc.tensor.matmul(v_ps[:], posT[0:2, :], aaug[0:2, :],
                             start=True, stop=True)
            # scatter u/v scene blocks into the combined uv tiles
            # (u copies on ACT, v copies on DVE -- parallel chains)
            for s in range(SC):
                q, j = divmod(s, 2)
                nc.scalar.copy(uv[q][64 * j:64 * j + P, :],
                               u_ps[32 * s:32 * s + P, :])
                nc.vector.tensor_copy(uv[q][64 * j + 32:64 * j + 32 + P, :],
                                      v_ps[32 * s:32 * s + P, :])

            # ================= social pooling =================
            # layer-1 grid (one K=56 matmul per chunk/scene/half) interleaved
            # with layer-2 windows: after scene s's relus land, the 504-wide
            # windows that only touch scenes <= s are emitted, so the PE
            # stays dense while the next scene's relu chain runs on ACT.
            praw = [wpool.tile([128, 96], F32, name=f"praw{m}") for m in range(8)]
            NW = [(0, 504), (504, 504), (1008, 504), (1512, 504), (2016, 288)]
            ready_w = {0: [0], 1: [1], 2: [2], 3: [3, 4]}  # scene -> windows

            def emit_y1(s, hh, k):
                q, j = divmod(s, 2)
                y_ps = ps1.tile([128, HP], F32, tag="p1", name="y_ps")
                nc.tensor.matmul(
                    y_ps[:],
                    uv[q][64 * j:64 * j + 56, k * 128:(k + 1) * 128],
                    jimat[64 * j:64 * j + 56, hh * HP:(hh + 1) * HP],
                    start=True, stop=True, tile_position=(64 * j, 0))
                nc.scalar.activation(
                    ybig[k][:, s * 576 + hh * HP:s * 576 + (hh + 1) * HP],
                    y_ps[:], AF.Relu)

            def emit_l2m(w, m):
                c0, cw = NW[w]
                l2 = psB.tile([128, 504], F32, tag="l2", name="l2")
                for k in range(4):
                    nc.tensor.matmul(
                        l2[:, 0:cw],
                        wp2[:, k * BNK + m * 128:k * BNK + (m + 1) * 128],
                        ybig[k][:, c0:c0 + cw],
                        start=(k == 0), stop=(k == 3))
                nc.vector.tensor_reduce(
                    praw[m][:, c0 // P:(c0 + cw) // P],
                    l2[:, 0:cw].rearrange("p (i j) -> p i j", j=P),
                    axis=mybir.AxisListType.X, op=OP.max)

            for hh in range(2):
                for k in range(4):
                    emit_y1(0, hh, k)
            for s in range(1, SC):
                w = ready_w[s - 1][0]
                for i, (hh, k) in enumerate([(a, b) for a in range(2)
                                             for b in range(4)]):
                    emit_y1(s, hh, k)
                    emit_l2m(w, i)
            for w in ready_w[SC - 1]:
                for m in range(8):
                    emit_l2m(w, m)
            for m in range(8):
                nc.scalar.activation(
                    poolT[m][:].rearrange("p (b c) -> p b c", c=32)[:, :, 0:P],
                    praw[m][:].rearrange("p (b c) -> p b c", c=P),
                    AF.Relu, bias=bp2[:, m:m + 1])

            # ================= decoder MLP =================
            out1 = wpool.tile([PD, MLP], F32, name="out1")
            for n in range(2):
                d1 = ps1.tile([PD, 512], F32, tag="p1", name="d1")
                nc.tensor.matmul(d1[:], h_newT[:],
                                 wm1[:, n * 512:n * 512 + 512],
                                 start=True, stop=False)
                for k in range(8):
                    nc.tensor.matmul(
                        d1[:], poolT[k][:],
                        wm1[:, (k + 1) * MLP + n * 512:(k + 1) * MLP + n * 512 + 512],
                        start=False, stop=False)
                nc.tensor.matmul(d1[:], ones[:], wm1b[:, n * 512:(n + 1) * 512],
                                 start=False, stop=True)
                nc.scalar.activation(out1[:, n * 512:(n + 1) * 512], d1[:], AF.Relu)
            d2 = psB.tile([PD, 504], F32, tag="l2", name="d2")
            dd = d2[:, 0:H]
            nc.tensor.matmul(dd, f32(ones[:]), wm2b[:], start=True, stop=False)
            for k in range(8):
                trk = ps1.tile([PD, PD], F32, tag="p1", name="trk")
                nc.tensor.transpose(trk[:], out1[:, k * 128:(k + 1) * 128], ident[:])
                o1t = ypool.tile([128, PD], F32, tag="ysb", name="o1t")
                nc.scalar.copy(o1t[:], trk[:])
                nc.tensor.matmul(dd, o1t[:], wm2[:, k * H:(k + 1) * H],
                                 start=False, stop=(k == 7))
            hfin = wpool.tile([PD, H], F32, name="hfin")
            nc.scalar.activation(hfin[:], dd, AF.Relu)
            trh = ps1.tile([PD, PD], F32, tag="p1", name="trh")
            nc.tensor.transpose(trh[:], hfin[:], ident[:])
            nc.scalar.copy(hT[:], trh[:, :])

        dma(hfinT_d[:], f32(hT[:]))

    nc.compile()
    return nc


_PROGRAM_CACHE = {}


def _get_program(steps=T):
    if steps not in _PROGRAM_CACHE:
        _PROGRAM_CACHE[steps] = build_program(steps)
    return _PROGRAM_CACHE[steps]


def _host_prep(inputs):
    """Host-side weight precompute + per-core sharding. Returns in_maps."""
    f = lambda k: np.asarray(inputs[k], np.float32)
    W_se, b_se = f("W_se"), f("b_se")
    W_ih, b_ih, W_hh, b_hh = f("W_ih"), f("b_ih"), f("W_hh"), f("b_hh")
    W_hp, b_hp = f("W_hp"), f("b_hp")
    W_pse, b_pse = f("W_pse"), f("b_pse")
    W_p1, b_p1, W_p2, b_p2 = f("W_p1"), f("b_p1"), f("W_p2"), f("b_p2")
    W_m1, b_m1, W_m2, b_m2 = f("W_m1"), f("b_m1"), f("W_m2"), f("b_m2")
    last_pos, last_pos_rel = f("last_pos"), f("last_pos_rel")
    h0, c0 = f("h0"), f("c0")

    A = W_pse @ W_p1[:E]
    b1 = b_pse @ W_p1[:E] + b_p1
    shared = {
        "wih": np.concatenate([W_ih, (b_ih + b_hh)[None]], 0),
        "whh": W_hh,
        "whp": W_hp,
        "bhp": b_hp[:, None].copy(),
        "wse": np.concatenate([W_se, b_se[None]], 0),
        "aaug": np.concatenate([A, b1[None]], 0),
        "wh1": W_p1[E:].copy(),
        "wp2": W_p2,
        "bp2": b_p2[:, None].copy(),
        "wm1": np.concatenate([W_m1, b_m1[None]], 0),
        "wm2": np.concatenate([W_m2, b_m2[None]], 0),
        "ident": np.eye(PD, dtype=np.float32),
    }
    # combined selector matrix: per 64-row block, rows 0:24 pick u_j, rows
    # 32:56 subtract v_i; column c = h*288 + i_local*24 + j
    JI = np.zeros((64, 2 * HP), np.float32)
    for h in range(2):
        for i_l in range(12):
            for jj in range(P):
                JI[jj, h * HP + i_l * P + jj] = 1.0
                JI[32 + 12 * h + i_l, h * HP + i_l * P + jj] = -1.0
    shared["jimat"] = np.concatenate([JI, JI], 0)

    dec0 = last_pos_rel @ W_se + b_se    # (B, E)

    def pad_pedsT(x):  # (n_peds_core, D) -> (D, PD) scene-blocked
        D = x.shape[1]
        out = np.zeros((D, PD), np.float32)
        for s in range(SC):
            out[:, 32 * s:32 * s + P] = x[s * P:(s + 1) * P].T
        return out

    in_maps = []
    for core in range(N_CORES):
        sl = slice(core * SC * P, (core + 1) * SC * P)
        m = dict(shared)
        m["h0T"] = pad_pedsT(h0[0, sl])
        m["c0p"] = pad_pedsT(c0[0, sl]).T.copy()
        m["lposT"] = pad_pedsT(last_pos[sl])[:2].copy()
        m["dec0T"] = pad_pedsT(dec0[sl])
        in_maps.append(m)
    return in_maps


def _unpack(results):
    pred = np.zeros((T, B, 2), np.float32)
    hfin = np.zeros((1, B, H), np.float32)
    for core in range(N_CORES):
        pT = results[core]["predT"]   # (T, 2, PD)
        hT = results[core]["hfinT"]   # (H, PD)
        for s in range(SC):
            gsl = slice((core * SC + s) * P, (core * SC + s + 1) * P)
            psl = slice(32 * s, 32 * s + P)
            pred[:, gsl, :] = pT[:, :, psl].transpose(0, 2, 1)
            hfin[0, gsl, :] = hT[:, psl].T
    return pred, hfin


def kernel(**inputs):
    nc = _get_program()
    in_maps = _host_prep(inputs)
    res = run_bass_kernel_spmd(nc, in_maps, list(range(N_CORES)))
    return _unpack(res.results)
